# revision 1
# baseline (speedup 1.0000x reference)
"""Trainium2 Bass kernel for nn_BertAdapterCapsuleMask.

Strategy (8 NeuronCores, SPMD — identical program, per-core data):

The reference computes, per example b:
  sem   = squash_n(x @ sem_w + sem_b)                      (capsule layer)
  priors[c,n,:] = sem[n,:] @ route_weights[c,n]            (routing priors)
  vote  = 3-iter masked dynamic routing over (c,b) pairs
  h_out = reshape(vote,(B,S,C)) @ larger_w' + larger_b'    (NB: the reshape
          mixes examples: h_out[b] reads vote rows 3b..3b+2 of the
          row-major [C*B, S] vote matrix)
  out   = x + adapter(x + h_out)                           (768->2048->768 MLP)

Sharding: the routing problem is independent per (c,b) pair (384 pairs).
Core k owns pairs t in [48k, 48k+48) AND examples b in [16k, 16k+16).
Because vote row index used by h_out[b] is exactly 3b..3b+2, core k's own
pairs produce precisely the vote rows its own examples need — zero
cross-core communication.  Each core computes sem for the 48 examples
b' = t mod 128 its pairs reference (sem is cheap), then routing, then the
adapter for its 16 own examples.

Weight folds (host side, exact):
  gfc1 folded into fc2_w rows;  glarger into larger_w;  (larger_b*glarger)
  and larger_b's path folded into fc1_b;  h_out@fc1_w folded to
  V @ M1 with M1 = (larger_w*glarger) @ fc1_w, so h_out never materializes.

Precision: matmuls in bf16 (fp32 PSUM accumulation), routing arithmetic in
fp32, final residual adds the untouched fp32 x.
"""

import numpy as np
import ml_dtypes

import concourse.bass as bass
import concourse.bacc as bacc
import concourse.mybir as mybir
import concourse.tile as tile
from concourse import bass_utils

BF16 = ml_dtypes.bfloat16
F32 = mybir.dt.float32
BF = mybir.dt.bfloat16
AF = mybir.ActivationFunctionType
ALU = mybir.AluOpType

B, S, H, A, C, N = 128, 128, 768, 2048, 3, 10
NUM_ITER = 3
NCORES = 8
BL = B // NCORES          # 16 own examples / core
NPAIR = 3 * B // NCORES   # 48 routing pairs / core
HK = H // 128             # 6
AK = A // 128             # 16
TOK = BL * S              # 2048 tokens / core
HALF = TOK // 2           # 1024
NC30 = N * C              # 30
NSEM = 18                 # sem examples per core: [16k, 16k+18) mod 128


def _sigmoid_f32(z):
    z = np.asarray(z, np.float32)
    out = np.empty_like(z)
    pos = z >= 0
    out[pos] = 1.0 / (1.0 + np.exp(-z[pos], dtype=np.float32))
    ez = np.exp(z[~pos], dtype=np.float32)
    out[~pos] = ez / (1.0 + ez)
    return out.astype(np.float32)


def _bf(x):
    return np.ascontiguousarray(np.asarray(x, np.float32).astype(BF16))


# ---------------------------------------------------------------------------
# device program
# ---------------------------------------------------------------------------

def _build_program(act_n, variant="full"):
    """variant: 'full' | 'capsule' (skip adapter, copy x->out) |
    'adapter' (skip capsule phases, memset vt) | 'p1'/'p2'/'p3' (capsule
    prefixes: sem only / +squash / +priors)."""
    level = {"p1": 1, "p2": 2, "p3": 3, "capsule": 4, "full": 4, "adapter": 0}[variant]
    nc = bacc.Bacc("TRN2", target_bir_lowering=False, debug=False,
                   num_devices=NCORES)

    d_xtsem = nc.dram_tensor("xt_sem", [6, HK, 128, 3 * S], F32, kind="ExternalInput")
    d_xtown = nc.dram_tensor("xt_own", [HK, 128, TOK], BF, kind="ExternalInput")
    d_xown = nc.dram_tensor("x_own", [BL, S, H], F32, kind="ExternalInput")
    d_rw = nc.dram_tensor("rw_pack", [C, 128, act_n * C * S], F32, kind="ExternalInput")
    d_sw = nc.dram_tensor("sw", [HK, 128, NC30], F32, kind="ExternalInput")
    d_semb = nc.dram_tensor("semb", [1, NC30], F32, kind="ExternalInput")
    d_m1 = nc.dram_tensor("m1", [C, A], BF, kind="ExternalInput")
    d_fc1w = nc.dram_tensor("fc1w", [HK, 128, A], BF, kind="ExternalInput")
    d_fc1b = nc.dram_tensor("fc1b", [128, AK], F32, kind="ExternalInput")
    d_fc2w = nc.dram_tensor("fc2w", [AK, 128, H], BF, kind="ExternalInput")
    d_b2 = nc.dram_tensor("b2row", [1, H], BF, kind="ExternalInput")
    d_g2 = nc.dram_tensor("g2", [1, H], F32, kind="ExternalInput")
    d_masks = nc.dram_tensor("masks", [NPAIR, C], F32, kind="ExternalInput")
    d_vcb = nc.dram_tensor("votecb", [NPAIR * S], BF, kind="Internal")
    d_out = nc.dram_tensor("out", [BL, S, H], F32, kind="ExternalOutput")

    with tile.TileContext(nc) as tc:
        with (
            tc.tile_pool(name="w", bufs=1) as wp,
            tc.tile_pool(name="semx", bufs=2) as sxp,
            tc.tile_pool(name="sem", bufs=1) as smp,
            tc.tile_pool(name="rt", bufs=1) as rp,
            tc.tile_pool(name="ad", bufs=1) as ap_,
            tc.tile_pool(name="st", bufs=2) as sp,
            tc.tile_pool(name="ps", bufs=8, space="PSUM") as pp,
            tc.tile_pool(name="dram", bufs=1, space="DRAM") as dp,
        ):
            # ---------------- persistent weights -----------------
            # (adapter-prepass inputs first: PE can start on fc1 immediately)
            xo_sb = wp.tile([128, HK * TOK], BF, tag="bigx", bufs=1)
            for hk in range(HK):
                nc.scalar.dma_start(xo_sb[:, hk * TOK:(hk + 1) * TOK], d_xtown[hk])
            fc1w_sb = wp.tile([128, HK * A], BF)
            for hk in range(HK):
                nc.scalar.dma_start(fc1w_sb[:, hk * A:(hk + 1) * A], d_fc1w[hk])
            fc1b_sb = wp.tile([128, AK], F32)
            nc.scalar.dma_start(fc1b_sb[:], d_fc1b[:])
            sw_sb = wp.tile([128, HK * NC30], F32)
            for hk in range(HK):
                nc.sync.dma_start(sw_sb[:, hk * NC30:(hk + 1) * NC30], d_sw[hk])
            semb_sb = wp.tile([1, NC30], F32)
            nc.sync.dma_start(semb_sb[:], d_semb[:])
            ones_sb = wp.tile([1, 128], BF)
            nc.gpsimd.memset(ones_sb[:], 1.0)
            ones_f = wp.tile([1, 128], F32)
            nc.gpsimd.memset(ones_f[:], 1.0)
            masks_sb = wp.tile([NPAIR, C], F32)
            nc.sync.dma_start(masks_sb[:], d_masks[:])
            m1_sb = wp.tile([C, A], BF)
            nc.sync.dma_start(m1_sb[:], d_m1[:])
            fc2w_sb = wp.tile([128, AK * H], BF, tag="bigx", bufs=1)
            for ak in range(AK):
                nc.scalar.dma_start(fc2w_sb[:, ak * H:(ak + 1) * H], d_fc2w[ak])
            b2_sb = wp.tile([1, H], BF)
            nc.sync.dma_start(b2_sb[:], d_b2[:])
            g2rep = wp.tile([128, H], F32)
            g2_src = d_g2.ap()  # [1, H] dram -> broadcast to 128 partitions
            g2_b = bass.AP(g2_src.tensor, g2_src.offset, [[0, 128], [1, H]])
            nc.sync.dma_start(g2rep[:], g2_b)

            # ---------------- fc1 pass 1 (x-only part; no routing dep) -----
            # z1p accumulates fc1_w.T @ xT; the capsule term M1.T@VT, bias and
            # relu are applied in pass 2 once routing is done.  Half B is
            # emitted after fc2-A (its z1p slot reuses half A's).
            z1ps = {}

            def emit_fc1_pass1(hf):
                z1p = ap_.tile([128, AK * HALF], BF, tag="z1p", bufs=2,
                               name=f"z1p_{hf}")
                z1ps[hf] = z1p
                for ak in range(AK):
                    pss = [pp.tile([128, 512], F32, tag="mm",
                                   name=f"ps_p1_{hf}_{ak}_{i}") for i in range(2)]
                    for hk in range(HK):
                        lhsT = fc1w_sb[:, hk * A + ak * 128: hk * A + (ak + 1) * 128]
                        for i in range(2):
                            col = hf * HALF + i * 512
                            nc.tensor.matmul(
                                pss[i][:], lhsT,
                                xo_sb[:, hk * TOK + col: hk * TOK + col + 512],
                                start=(hk == 0), stop=(hk == HK - 1))
                    for i in range(2):
                        nc.scalar.copy(
                            z1p[:, ak * HALF + i * 512: ak * HALF + (i + 1) * 512],
                            pss[i][:])


            if variant != "adapter":
                # ------- phase 1: sem + squash (18 examples [16k,16k+18)) -----
                sem_own = smp.tile([128, NSEM * NC30], F32)
                for g in range(6):
                    xt_g = sxp.tile([128, HK * 3 * S], F32, tag="xtg")
                    src_ = d_xtsem.ap()[g]  # [HK, 128, 384]
                    nc.sync.dma_start(
                        xt_g[:].rearrange("p (hk c) -> p hk c", hk=HK),
                        src_.rearrange("hk p c -> p hk c"))
                    for el in range(3):
                        slot = g * 3 + el
                        ps = pp.tile([128, NC30], F32, tag="mm", name=f"ps_sem_{slot}")
                        for hk in range(HK):
                            nc.tensor.matmul(
                                ps[:],
                                xt_g[:, hk * (3 * S) + el * S: hk * (3 * S) + (el + 1) * S],
                                sw_sb[:, hk * NC30:(hk + 1) * NC30],
                                start=(hk == 0), stop=False)
                        nc.tensor.matmul(ps[:], ones_f[:], semb_sb[:],
                                         start=False, stop=True)
                        nc.scalar.copy(sem_own[:, slot * NC30:(slot + 1) * NC30], ps[:])

                # squash over n:  f = sqrt(sq)/(1+sq) via exp(0.5*ln(sq))
                sem2 = smp.tile([128, NSEM * NC30], F32)
                nc.vector.tensor_tensor(sem2[:], sem_own[:], sem_own[:], op=ALU.mult)
                sqt = smp.tile([128, NSEM * C], F32)
                nc.vector.tensor_reduce(
                    sqt[:].rearrange("p (slot cc) -> p slot cc", cc=C),
                    sem2[:].rearrange("p (slot n cc) -> p slot cc n", n=N, cc=C),
                    axis=mybir.AxisListType.X, op=ALU.add)
                lnq = smp.tile([128, NSEM * C], F32)
                nc.scalar.activation(lnq[:], sqt[:], AF.Ln)
                sqq = smp.tile([128, NSEM * C], F32)
                nc.scalar.activation(sqq[:], lnq[:], AF.Exp, scale=0.5)  # sqrt(sq)
                up = smp.tile([128, NSEM * C], F32)
                nc.vector.tensor_scalar_add(up[:], sqt[:], 1.0)
                ru = smp.tile([128, NSEM * C], F32)
                nc.vector.reciprocal(ru[:], up[:])
                fq = smp.tile([128, NSEM * C], F32)
                nc.vector.tensor_tensor(fq[:], sqq[:], ru[:], op=ALU.mult)
                # sem_sq = sem_own * f  (f broadcast over n), fp32
                sem_sq = sem2  # reuse scratch
                f_ap = fq[:]
                f_b = bass.AP(f_ap.tensor, f_ap.offset,
                              [f_ap.ap[0], [C, NSEM], [0, N], [1, C]])
                nc.vector.tensor_tensor(
                    sem_sq[:].rearrange("p (slot n cc) -> p slot n cc", n=N, cc=C),
                    sem_own[:].rearrange("p (slot n cc) -> p slot n cc", n=N, cc=C),
                    f_b, op=ALU.mult)
                # materialize pair-ordered copy: block p=3i+u <- slot i+u
                # (matmul weight APs allow only one free dim, so gather here)
                sem_pair = smp.tile([128, NPAIR * NC30], F32)
                sq_ap = sem_sq[:]
                gather = bass.AP(sq_ap.tensor, sq_ap.offset,
                                 [sq_ap.ap[0], [NC30, BL], [NC30, C], [1, NC30]])
                nc.vector.tensor_copy(
                    sem_pair[:].rearrange("p (i u nc) -> p i u nc", i=BL, u=C),
                    gather)

                if level >= 3:
                    # ---------------- phase 2: priors -----------------
                    # lhsT rows (pair p = 3i+u) read sem slot i+u:
                    # AP dims [(30,16)@i, (30,3)@u] both stride 30 (overlapping)
                    sem_v = sem_pair[:].rearrange("p (pair nc) -> p nc pair", nc=NC30)
                    priors = rp.tile([NPAIR, act_n * S], F32)
                    for g in range(C):
                        for n in range(act_n):
                            ps = pp.tile([NPAIR, S], F32, tag="mm", name=f"ps_pr_{g}_{n}")
                            rwt = sxp.tile([128, C * S], F32, tag="rwt", bufs=4,
                                           name=f"rw_{g}_{n}")
                            nc.scalar.dma_start(
                                rwt[:], d_rw.ap()[g][:, (n * C) * S:(n * C + C) * S])
                            for cc in range(C):
                                nc.tensor.matmul(
                                    ps[:], sem_v[:, n * C + cc, :],
                                    rwt[:, cc * S:(cc + 1) * S],
                                    start=(cc == 0), stop=(cc == C - 1))
                            dst = priors[:, n * S:(n + 1) * S]
                            for g2 in range(1):
                                pass
                            if g == 0:
                                nc.vector.tensor_scalar_mul(dst, ps[:], masks_sb[:, 0:1])
                            else:
                                nc.vector.scalar_tensor_tensor(
                                    dst, ps[:], masks_sb[:, g:g + 1], dst,
                                    op0=ALU.mult, op1=ALU.add)

                if level >= 4:
                    # ---------------- phase 3: routing -----------------
                    vote = rp.tile([NPAIR, S], F32)
                    scr = rp.tile([NPAIR, S], F32)
                    La = rp.tile([NPAIR, act_n], F32)
                    Lb = rp.tile([NPAIR, act_n], F32)
                    sqv = rp.tile([NPAIR, 1], F32)
                    lv = rp.tile([NPAIR, 1], F32)
                    sv = rp.tile([NPAIR, 1], F32)
                    uv = rp.tile([NPAIR, 1], F32)
                    rv = rp.tile([NPAIR, 1], F32)
                    fv = rp.tile([NPAIR, 1], F32)
                    outv = rp.tile([NPAIR, S], F32)
                    mx = rp.tile([NPAIR, 1], F32)
                    mneg = rp.tile([NPAIR, 1], F32)
                    ex = rp.tile([NPAIR, act_n], F32)
                    es = rp.tile([NPAIR, 1], F32)
                    ers = rp.tile([NPAIR, 1], F32)
                    probs = rp.tile([NPAIR, act_n], F32)

                    def vote_from(pr_scalar_ap_or_const, first_const=None):
                        """vote = sum_n probs_n * priors_n."""
                        for n in range(act_n):
                            blk = priors[:, n * S:(n + 1) * S]
                            sc = (first_const if first_const is not None
                                  else pr_scalar_ap_or_const[:, n:n + 1])
                            if n == 0:
                                nc.vector.tensor_scalar_mul(vote[:], blk, sc)
                            else:
                                nc.vector.scalar_tensor_tensor(
                                    vote[:], blk, sc, vote[:], op0=ALU.mult, op1=ALU.add)

                    def squash_vote():
                        nc.vector.tensor_tensor(scr[:], vote[:], vote[:], op=ALU.mult)
                        nc.vector.tensor_reduce(sqv[:], scr[:],
                                                axis=mybir.AxisListType.X, op=ALU.add)
                        nc.scalar.activation(lv[:], sqv[:], AF.Ln)
                        nc.scalar.activation(sv[:], lv[:], AF.Exp, scale=0.5)
                        nc.vector.tensor_scalar_add(uv[:], sqv[:], 1.0)
                        nc.vector.reciprocal(rv[:], uv[:])
                        nc.vector.tensor_tensor(fv[:], sv[:], rv[:], op=ALU.mult)
                        nc.vector.tensor_scalar_mul(outv[:], vote[:], fv[:])

                    def deltas(Lprev, Lnew, first):
                        for n in range(act_n):
                            nc.vector.tensor_tensor(
                                scr[:], priors[:, n * S:(n + 1) * S], outv[:],
                                op=ALU.mult)
                            nc.vector.tensor_reduce(
                                Lnew[:, n:n + 1], scr[:],
                                axis=mybir.AxisListType.X, op=ALU.add)
                        if not first:
                            nc.vector.tensor_tensor(Lnew[:], Lnew[:], Lprev[:],
                                                    op=ALU.add)

                    def softmax(L):
                        nc.vector.tensor_reduce(mx[:], L[:], axis=mybir.AxisListType.X,
                                                op=ALU.max)
                        nc.vector.tensor_scalar_mul(mneg[:], mx[:], -1.0)
                        nc.scalar.activation(ex[:], L[:], AF.Exp, bias=mneg[:],
                                             accum_out=es[:])
                        nc.vector.reciprocal(ers[:], es[:])
                        nc.vector.tensor_scalar_mul(probs[:], ex[:], ers[:])

                    # iter 0
                    vote_from(None, first_const=1.0 / act_n)
                    squash_vote()
                    deltas(None, La, first=True)
                    # iter 1
                    softmax(La)
                    vote_from(probs)
                    squash_vote()
                    deltas(La, Lb, first=False)
                    # iter 2 (final)
                    softmax(Lb)
                    vote_from(probs)

                    vb = rp.tile([NPAIR, S], BF)
                    nc.vector.tensor_copy(vb[:], vote[:])
                    nc.sync.dma_start(
                        d_vcb.ap().rearrange("(p s) -> p s", p=NPAIR), vb[:])

                    # VT[c, e*128+s] = votecb_flat[3*e*128 + 3*s + c]
                    vt_sb = ap_.tile([C, TOK], BF)
                    vflat = d_vcb.ap()
                    for e in range(BL):
                        src = bass.AP(vflat.tensor, vflat.offset + 3 * e * S,
                                      [[1, C], [C, S]])
                        nc.sync.dma_start(vt_sb[:, e * S:(e + 1) * S], src)

            else:
                vt_sb = ap_.tile([C, TOK], BF)
                nc.gpsimd.memset(vt_sb[:], 0.0)

            if variant in ("full", "adapter"):
                emit_fc1_pass1(0)
                emit_fc1_pass1(1)
                # -------- phase 4: fc1 pass 2 (capsule term) + fc2 --------
                def emit_fc1_pass2_and_fc2(hf):
                    z1 = z1ps[hf]
                    for ak in range(AK):
                        ps2 = [pp.tile([128, 512], F32, tag="mm",
                                       name=f"ps_p2_{hf}_{ak}_{i}") for i in range(2)]
                        m1l = m1_sb[:, ak * 128:(ak + 1) * 128]
                        for i in range(2):
                            col = hf * HALF + i * 512
                            nc.tensor.matmul(ps2[i][:], m1l,
                                             vt_sb[:, col:col + 512],
                                             start=True, stop=True)
                        tmp = sp.tile([128, HALF], F32, tag="tmp",
                                      name=f"tmp_{hf}_{ak}", bufs=2)
                        for i in range(2):
                            # tmp = (ps2 + fc1b) + z1p
                            nc.vector.scalar_tensor_tensor(
                                tmp[:, i * 512:(i + 1) * 512], ps2[i][:],
                                fc1b_sb[:, ak:ak + 1],
                                z1[:, ak * HALF + i * 512: ak * HALF + (i + 1) * 512],
                                op0=ALU.add, op1=ALU.add)
                        nc.scalar.activation(
                            z1[:, ak * HALF:(ak + 1) * HALF], tmp[:], AF.Relu)
                    for tt in range(8):
                        e = hf * 8 + tt
                        psa = pp.tile([128, 512], F32, tag="mm", name=f"ps_f2a_{e}")
                        psb = pp.tile([128, 256], F32, tag="mm", name=f"ps_f2b_{e}")
                        for ak in range(AK):
                            lhsT = z1[:, ak * HALF + tt * 128: ak * HALF + (tt + 1) * 128]
                            nc.tensor.matmul(psa[:], lhsT,
                                             fc2w_sb[:, ak * H: ak * H + 512],
                                             start=(ak == 0), stop=False)
                            nc.tensor.matmul(psb[:], lhsT,
                                             fc2w_sb[:, ak * H + 512: ak * H + H],
                                             start=(ak == 0), stop=False)
                        nc.tensor.matmul(psa[:], ones_sb[:], b2_sb[:, 0:512],
                                         start=False, stop=True)
                        nc.tensor.matmul(psb[:], ones_sb[:], b2_sb[:, 512:H],
                                         start=False, stop=True)
                        xt = sp.tile([128, H], F32, tag="x", name=f"x_{e}")
                        nc.sync.dma_start(xt[:], d_xown[e])
                        ot = sp.tile([128, H], F32, tag="o", name=f"o_{e}")
                        nc.scalar.activation(ot[:, 0:512], psa[:], AF.Relu)
                        nc.scalar.activation(ot[:, 512:H], psb[:], AF.Relu)
                        nc.vector.tensor_tensor(ot[:], ot[:], g2rep[:], op=ALU.mult)
                        nc.vector.tensor_tensor(ot[:], ot[:], xt[:], op=ALU.add)
                        nc.sync.dma_start(d_out[e], ot[:])

                emit_fc1_pass2_and_fc2(0)
                emit_fc1_pass2_and_fc2(1)
            else:
                for e in range(BL):
                    xt = sp.tile([128, H], F32, tag="x", name=f"xc_{e}")
                    nc.sync.dma_start(xt[:], d_xown[e])
                    nc.sync.dma_start(d_out[e], xt[:])

    nc.compile()
    return nc


# ---------------------------------------------------------------------------
# host marshaling
# ---------------------------------------------------------------------------

def _prep_core_inputs(k, x, shared, act_n):
    # own (output) examples: b_i = 48k + 43 i (mod 128).  Because
    # 3*43 = 129 = 1 (mod 128), the 48 routing pairs t = 3 b_i + u map to
    # sem examples b' = t mod 128 = 16k + (i + u) mod 128 — just the 18
    # consecutive examples [16k, 16k+18).  Pair (i,u) sits at row 3i+u and
    # reads sem slot i+u; votecb rows 3e..3e+2 are exactly what h_out of
    # own example e needs, so no cross-core traffic anywhere.
    own = np.array([(48 * k + 43 * i) % B for i in range(BL)])
    sem_ex = np.array([(16 * k + j) % B for j in range(NSEM)])

    # xt_sem: [6, hk, 128, 3*S] fp32, groups of 3 sem examples
    xs = np.transpose(x[sem_ex], (2, 0, 1)).reshape(H, NSEM * S).astype(np.float32)
    xt_sem = np.empty((6, HK, 128, 3 * S), np.float32)
    for g in range(6):
        for hk in range(HK):
            xt_sem[g, hk] = xs[hk * 128:(hk + 1) * 128,
                               g * 3 * S:(g + 1) * 3 * S]

    xo = np.transpose(x[own], (2, 0, 1)).reshape(H, TOK).astype(BF16)
    xt_own = np.ascontiguousarray(xo.reshape(HK, 128, TOK))
    x_own = np.ascontiguousarray(x[own].astype(np.float32))

    # group g == c' directly; mask[p, g] = (c' of pair p == g)
    masks = np.zeros((NPAIR, C), np.float32)
    for i in range(BL):
        for u in range(C):
            t = 3 * int(own[i]) + u
            masks[3 * i + u, t // B] = 1.0

    return {
        "xt_sem": xt_sem,
        "xt_own": xt_own,
        "x_own": x_own,
        "rw_pack": shared["rw_pack"],
        "masks": masks,
        **{n: shared[n] for n in ("sw", "semb", "m1", "fc1w", "fc1b",
                                  "fc2w", "b2row", "g2")},
    }


_CACHE = {}


def _make_shared(inputs):
    fc1_w = np.asarray(inputs["fc1_w"], np.float32)
    fc1_b = np.asarray(inputs["fc1_b"], np.float32)
    fc2_w = np.asarray(inputs["fc2_w"], np.float32)
    fc2_b = np.asarray(inputs["fc2_b"], np.float32)
    efc1 = np.asarray(inputs["efc1"], np.float32)
    efc2 = np.asarray(inputs["efc2"], np.float32)
    sem_w = np.asarray(inputs["sem_w"], np.float32)
    sem_b = np.asarray(inputs["sem_b"], np.float32)
    route_weights = np.asarray(inputs["route_weights"], np.float32)
    larger_w = np.asarray(inputs["larger_w"], np.float32)
    larger_b = np.asarray(inputs["larger_b"], np.float32)
    elarger = np.asarray(inputs["elarger"], np.float32)
    t = int(np.asarray(inputs["t"]))
    sf = np.float32(int(np.asarray(inputs["s"])))
    act_n = t + 1

    gfc1 = _sigmoid_f32(sf * efc1[t])
    gfc2 = _sigmoid_f32(sf * efc2[t])
    glarger = _sigmoid_f32(sf * elarger[t])

    lwg = (larger_w * glarger[None, :]).astype(np.float32)
    lb_eff = (larger_b * glarger).astype(np.float32)
    rw4 = route_weights.reshape(C, N, S, C, S)

    return {
        "sw": np.ascontiguousarray(np.transpose(sem_w, (1, 0, 2))
                                   .reshape(H, NC30).astype(np.float32)
                                   ).reshape(HK, 128, NC30),
        "semb": np.ascontiguousarray(sem_b.reshape(1, NC30).astype(np.float32)),
        "m1": _bf(lwg @ fc1_w),
        "fc1w": _bf(fc1_w).reshape(HK, 128, A),
        "fc1b": np.ascontiguousarray(
            (fc1_b + lb_eff @ fc1_w).astype(np.float32).reshape(AK, 128).T),
        "fc2w": _bf(fc2_w * gfc1[:, None]).reshape(AK, 128, H),
        "b2row": _bf(fc2_b.reshape(1, H)),
        "g2": np.ascontiguousarray(gfc2.reshape(1, H)),
        "rw_pack": np.stack([
            np.ascontiguousarray(np.transpose(rw4[c, :act_n], (1, 0, 2, 3))
                                 .reshape(S, act_n * C * S).astype(np.float32))
            for c in range(C)]),
    }


def kernel(**inputs):
    x = np.asarray(inputs["x"], np.float32)
    t = int(np.asarray(inputs["t"]))
    act_n = t + 1
    shared = _make_shared(inputs)

    if act_n not in _CACHE:
        _CACHE[act_n] = _build_program(act_n)
    nc = _CACHE[act_n]

    in_maps = [_prep_core_inputs(k, x, shared, act_n) for k in range(NCORES)]
    res = bass_utils.run_bass_kernel_spmd(nc, in_maps, core_ids=list(range(NCORES)))
    out = np.empty((B, S, H), np.float32)
    for k in range(NCORES):
        own = [(48 * k + 43 * i) % B for i in range(BL)]
        out[own] = res.results[k]["out"]
    return out



# revision 15
# speedup vs baseline: 2.1860x; 2.1860x over previous
"""Trainium2 Bass kernel for nn_BertAdapterCapsuleMask (v2).

Strategy (8 NeuronCores, SPMD, data-parallel over batch):

Core k owns examples b_i = (48k + 43i) mod 128 (i<16).  Their routing
pairs t = 3b+u are exactly vote rows [48k,48k+48) and reference sem
examples [16k,16k+18) (consecutive) — zero cross-core traffic.

Key speed levers vs the v1 kernel (297us):
  * adapter GEMMs (fc1/fc2/m1-term) run in fp8-e4m3 DoubleRow mode
    (2 k-subtiles per matmul, 0.5 cyc/out-column = 4x bf16 throughput).
    Weights are pre-scaled by 16 (and vt/m1 by 4) to dodge e4m3's
    denormal floor; scales are unwound in the psum->sbuf drains.
  * single-pass fc1: x-part, capsule (m1) part and bias accumulate in
    PSUM; one fused relu drain emits z1=16*relu(.) straight to fp8.
    This kills v1's z1p roundtrip (72us Act + 42us DVE).
  * sem/priors matmuls in fp16; priors produced directly in a
    [d=128, (n,pair)] psum so the 3-iter routing loop is ~30 small
    free-dim-billed vector ops + tiny helper matmuls (column reduction
    and partition replication via ones-matmuls).
  * drains/adds round-robin over Act/Pool/DVE; residual x and output
    travel as fp16 ([h, token] layout, transposed back on host).

Numerics (validated host-side): rel-err ~1.4e-2 vs the 2e-2 gate;
sem/priors fp16, routing fp32, adapters fp8, output fp16.
"""

import numpy as np
import ml_dtypes

import concourse.bass as bass
import concourse.bacc as bacc
import concourse.mybir as mybir
import concourse.tile as tile
from concourse import bass_utils

F8NP = ml_dtypes.float8_e4m3
F16NP = np.float16
F32 = mybir.dt.float32
F16 = mybir.dt.float16
F8 = mybir.dt.float8e4
AF = mybir.ActivationFunctionType
ALU = mybir.AluOpType
DR = mybir.MatmulPerfMode.DoubleRow

B, S, H, A, C, N = 128, 128, 768, 2048, 3, 10
NCORES = 8
BL = B // NCORES            # 16 own examples / core
NPAIR = 3 * BL              # 48 routing pairs / core
NSEM = 18                   # sem examples / core
TOK = BL * S                # 2048 own tokens / core
HK = H // 128               # 6
HKP = HK // 2               # 3 h double-chunks
AK = A // 128               # 16
AKP = AK // 2               # 8 a double-chunks
NC30 = N * C                # 30 sem cols / slot
NSEM_S = NSEM * S           # 2304


def _sigmoid_f32(z):
    z = np.asarray(z, np.float32)
    out = np.empty_like(z)
    pos = z >= 0
    out[pos] = 1.0 / (1.0 + np.exp(-z[pos], dtype=np.float32))
    ez = np.exp(z[~pos], dtype=np.float32)
    out[~pos] = ez / (1.0 + ez)
    return out.astype(np.float32)


def _f8(x):
    return np.ascontiguousarray(np.asarray(x, np.float32).astype(F8NP))


def _f16(x):
    return np.ascontiguousarray(np.asarray(x, np.float32).astype(F16NP))


# ---------------------------------------------------------------------------
# device program
# ---------------------------------------------------------------------------

def _build_program(act_n, dbg=False):
    an = act_n
    ANP = an * NPAIR            # routing free size (n-major, pair)
    nc = bacc.Bacc("TRN2", target_bir_lowering=False, debug=False,
                   num_devices=NCORES)

    d_sw = nc.dram_tensor("sw", [HK, 128, NC30], F16, kind="ExternalInput")
    d_semb = nc.dram_tensor("semb", [1, NC30], F16, kind="ExternalInput")
    d_xtsem = nc.dram_tensor("xtsem", [HK, 128, NSEM_S], F16, kind="ExternalInput")
    d_masks = nc.dram_tensor("masks", [128, 3 * NPAIR], F16, kind="ExternalInput")
    d_rw = nc.dram_tensor("rw", [C, 128, an * C * S], F16, kind="ExternalInput")
    d_ident = nc.dram_tensor("ident", [128, 128], F32, kind="ExternalInput")
    d_m1 = nc.dram_tensor("m1", [2, 2 * A], F8, kind="ExternalInput")
    d_b1 = nc.dram_tensor("b1", [128, AK], F32, kind="ExternalInput")
    d_b2a = nc.dram_tensor("b2a", [128, HK], F32, kind="ExternalInput")
    d_b2b = nc.dram_tensor("b2b", [128, HK], F32, kind="ExternalInput")
    d_xt8 = nc.dram_tensor("xt8", [HKP, 128, 2 * TOK], F8, kind="ExternalInput")
    d_fc1w = nc.dram_tensor("fc1w", [HKP, 128, 2 * A], F8, kind="ExternalInput")
    d_x16 = nc.dram_tensor("x16", [HK, 128, TOK], F16, kind="ExternalInput")
    d_fc2w = nc.dram_tensor("fc2w", [AKP, 128, 2 * H], F8, kind="ExternalInput")
    d_vcb = nc.dram_tensor("vcb", [NPAIR * S], F8, kind="Internal")
    d_out = nc.dram_tensor("out", [HK, 128, TOK], F16, kind="ExternalOutput")
    if dbg:
        d_dsem = nc.dram_tensor("dsem", [128, NSEM * NC30], F32,
                                kind="ExternalOutput")
        d_dsp = nc.dram_tensor("dsp", [128, NPAIR * an * C], F16,
                               kind="ExternalOutput")
        d_dpr = nc.dram_tensor("dpr", [128, ANP], F32, kind="ExternalOutput")
        d_dvote = nc.dram_tensor("dvote", [128, NPAIR], F32,
                                 kind="ExternalOutput")
        d_dz1 = nc.dram_tensor("dz1", [128, AK * TOK], F8,
                               kind="ExternalOutput")

    with tile.TileContext(nc) as tc:
        with (
            tc.tile_pool(name="w", bufs=1) as wp,
            tc.tile_pool(name="rt", bufs=1) as rp,
            tc.tile_pool(name="st", bufs=2) as sp,
            tc.tile_pool(name="ps", bufs=1, space="PSUM") as pp,
        ):
            # ---------------- DMAs: ordered by first use -----------------
            sw_sb = wp.tile([128, HK * NC30], F16)
            for hk in range(HK):
                nc.sync.dma_start(sw_sb[:, hk * NC30:(hk + 1) * NC30], d_sw[hk])
            semb_sb = wp.tile([1, NC30], F16)
            nc.sync.dma_start(semb_sb[:], d_semb[:])
            xtsem_sb = wp.tile([128, HK * NSEM_S], F16, tag="xts")
            for hk in range(HK):
                nc.sync.dma_start(
                    xtsem_sb[:, hk * NSEM_S:(hk + 1) * NSEM_S], d_xtsem[hk])
            masks_sb = wp.tile([128, 3 * NPAIR], F16)
            nc.sync.dma_start(masks_sb[:], d_masks[:])
            rw_sb = wp.tile([128, C * an * C * S], F16, tag="rw")
            for g in range(C):
                nc.sync.dma_start(
                    rw_sb[:, g * (an * C * S):(g + 1) * (an * C * S)], d_rw[g])
            ident_sb = wp.tile([128, 128], F32)
            nc.sync.dma_start(ident_sb[:], d_ident[:])
            m1_sb = wp.tile([2, 2 * A], F8)
            nc.sync.dma_start(m1_sb[:], d_m1[:])
            b1_sb = wp.tile([128, AK], F32)
            nc.sync.dma_start(b1_sb[:], d_b1[:])
            b2a_sb = wp.tile([128, HK], F32)
            nc.sync.dma_start(b2a_sb[:], d_b2a[:])
            b2b_sb = wp.tile([128, HK], F32)
            nc.sync.dma_start(b2b_sb[:], d_b2b[:])
            xt8_sb = wp.tile([128, HKP * 2 * TOK], F8, tag="xt8")
            for hp in range(HKP):
                nc.sync.dma_start(
                    xt8_sb[:, hp * 2 * TOK:(hp + 1) * 2 * TOK], d_xt8[hp])
            fc1w_sb = wp.tile([128, HKP * 2 * A], F8, tag="fc1w")
            for hp in range(HKP):
                nc.sync.dma_start(
                    fc1w_sb[:, hp * 2 * A:(hp + 1) * 2 * A], d_fc1w[hp])
            x16_sb = wp.tile([128, HK * TOK], F16, tag="x16")
            for hk in range(HK):
                nc.sync.dma_start(
                    x16_sb[:, hk * TOK:(hk + 1) * TOK], d_x16[hk])
            fc2w_sb = wp.tile([128, AKP * 2 * H], F8, tag="fc2w")
            for ap_ in range(AKP):
                nc.sync.dma_start(
                    fc2w_sb[:, ap_ * 2 * H:(ap_ + 1) * 2 * H], d_fc2w[ap_])

            # constants
            ones1_16 = wp.tile([1, 128], F16)
            nc.gpsimd.memset(ones1_16[:], 1.0)
            ones128 = wp.tile([128, 1], F32)
            nc.gpsimd.memset(ones128[:], 1.0)
            inv_an2 = wp.tile([1, 128], F32)
            nc.gpsimd.memset(inv_an2[:], 1.0 / (an * an))
            one_row = wp.tile([1, 128], F32)
            nc.gpsimd.memset(one_row[:], 1.0)
            vt2_sb = wp.tile([2, 2 * TOK], F8)
            nc.gpsimd.memset(vt2_sb[:], 0.0)

            # z1 lives across fc1->fc2
            z1_sb = wp.tile([128, AK * TOK], F8, tag="z1")

            # ---------------- phase 1: sem ([s,30] per slot) -------------
            sem_own = rp.tile([128, NSEM * NC30], F32)
            for half in range(2):
                ps = pp.tile([128, 9 * NC30], F32, tag="sm", bufs=2,
                             name=f"ps_sem_{half}")
                for j9 in range(9):
                    j = half * 9 + j9
                    dst = ps[:, j9 * NC30:(j9 + 1) * NC30]
                    for hk in range(HK):
                        nc.tensor.matmul(
                            dst,
                            xtsem_sb[:, hk * NSEM_S + j * S: hk * NSEM_S + (j + 1) * S],
                            sw_sb[:, hk * NC30:(hk + 1) * NC30],
                            start=(hk == 0), stop=False)
                    nc.tensor.matmul(dst, ones1_16[:], semb_sb[:],
                                     start=False, stop=True)
                nc.scalar.copy(
                    sem_own[:, half * 9 * NC30:(half + 1) * 9 * NC30], ps[:])

            # ---------------- phase 2: squash + sem_pair -----------------
            sem2 = rp.tile([128, NSEM * NC30], F32)
            nc.vector.tensor_tensor(sem2[:], sem_own[:], sem_own[:], op=ALU.mult)
            sqt = rp.tile([128, NSEM * C], F32)
            nc.vector.tensor_reduce(
                sqt[:].rearrange("p (slot cc) -> p slot cc", cc=C),
                sem2[:].rearrange("p (slot n cc) -> p slot cc n", n=N, cc=C),
                axis=mybir.AxisListType.X, op=ALU.add)
            lnq = rp.tile([128, NSEM * C], F32)
            nc.scalar.activation(lnq[:], sqt[:], AF.Ln)
            sqq = rp.tile([128, NSEM * C], F32)
            nc.scalar.activation(sqq[:], lnq[:], AF.Exp, scale=0.5)
            up = rp.tile([128, NSEM * C], F32)
            nc.vector.tensor_scalar_add(up[:], sqt[:], 1.0)
            ru = rp.tile([128, NSEM * C], F32)
            nc.vector.reciprocal(ru[:], up[:])
            fq = rp.tile([128, NSEM * C], F32)
            nc.vector.tensor_tensor(fq[:], sqq[:], ru[:], op=ALU.mult)
            # expand f to (slot, n, c) so the pair gather stays 3-dim
            fq18 = rp.tile([128, NSEM * an * C], F32)
            fqa = fq[:]
            fq_b = bass.AP(fqa.tensor, fqa.offset,
                           [fqa.ap[0], [C, NSEM], [0, an], [1, C]])
            nc.vector.tensor_copy(
                fq18[:].rearrange("p (slot n c) -> p slot n c", n=an, c=C),
                fq_b)

            # sem_pair[p, (pair, n, c)] = sem_own[s, (slot(pair), n, c)] * fq
            spair = rp.tile([128, NPAIR * an * C], F16)
            so = sem_own[:]
            gather = bass.AP(so.tensor, so.offset,
                             [so.ap[0], [NC30, BL], [NC30, C], [1, an * C]])
            f18 = fq18[:]
            fgather = bass.AP(f18.tensor, f18.offset,
                              [f18.ap[0], [an * C, BL], [an * C, C], [1, an * C]])
            nc.vector.tensor_tensor(
                spair[:].rearrange("p (i u nc) -> p i u nc", i=BL, u=C),
                gather, fgather, op=ALU.mult)

            # masked copies (one per rw group g)
            spg = rp.tile([128, 3 * NPAIR * an * C], F16)
            ms = masks_sb[:]
            for g in range(C):
                mask_b = bass.AP(ms.tensor, ms.offset + g * NPAIR,
                                 [ms.ap[0], [1, NPAIR], [0, an * C]])
                nc.vector.tensor_tensor(
                    spg[:, g * NPAIR * an * C:(g + 1) * NPAIR * an * C]
                    .rearrange("p (pair nc) -> p pair nc", nc=an * C),
                    spair[:].rearrange("p (pair nc) -> p pair nc", nc=an * C),
                    mask_b, op=ALU.mult)

            if dbg:
                nc.sync.dma_start(d_dsem[:], sem_own[:])
                nc.sync.dma_start(d_dsp[:], spair[:])

            # ---------------- phase 3: priors [d, (n, pair)] -------------
            ps_pr = pp.tile([128, ANP], F32, tag="pr", name="ps_pr")
            spg_ap = spg[:]
            for n in range(an):
                first = True
                for g in range(C):
                    for cc in range(C):
                        mov = bass.AP(
                            spg_ap.tensor,
                            spg_ap.offset + g * NPAIR * an * C + n * C + cc,
                            [spg_ap.ap[0], [an * C, NPAIR]])
                        nc.tensor.matmul(
                            ps_pr[:, n * NPAIR:(n + 1) * NPAIR],
                            rw_sb[:, g * an * C * S + n * C * S + cc * S:
                                  g * an * C * S + n * C * S + (cc + 1) * S],
                            mov,
                            start=first, stop=(g == C - 1 and cc == C - 1))
                        first = False

            if dbg:
                dbg_pr = rp.tile([128, ANP], F32)
                nc.scalar.copy(dbg_pr[:], ps_pr[:])
                nc.sync.dma_start(d_dpr[:], dbg_pr[:])

            # ---------------- phase 4: routing ---------------------------
            # priors view [p partitions(d), pair, n] with n innermost
            pr = ps_pr[:]
            pr_pn = bass.AP(pr.tensor, pr.offset,
                            [pr.ap[0], [1, NPAIR], [NPAIR, an]])

            vote = rp.tile([128, NPAIR], F32)
            scr48 = rp.tile([128, NPAIR], F32)
            outv = rp.tile([128, NPAIR], F32)
            scr288 = rp.tile([128, ANP], F32)
            lv = rp.tile([1, NPAIR], F32)
            sv = rp.tile([1, NPAIR], F32)
            uv = rp.tile([1, NPAIR], F32)
            rv = rp.tile([1, NPAIR], F32)
            fv = rp.tile([1, NPAIR], F32)
            mx = rp.tile([1, NPAIR], F32)
            l2 = rp.tile([1, ANP], F32)
            ex = rp.tile([1, ANP], F32)
            es = rp.tile([1, NPAIR], F32)
            ers = rp.tile([1, NPAIR], F32)
            probs16 = rp.tile([1, ANP], F16)

            ps_dl = pp.tile([1, ANP], F32, tag="dl", name="ps_dl")

            def squash_vote(it):
                # f = sqrt(sq)/(1+sq); it0 vote is an*true -> rescale consts
                nc.vector.tensor_tensor(scr48[:], vote[:], vote[:], op=ALU.mult)
                ps_sq = pp.tile([1, NPAIR], F32, tag="sm", bufs=2,
                                name=f"ps_sq_{it}")
                nc.tensor.matmul(ps_sq[:], ones128[:], scr48[:],
                                 start=True, stop=True)
                nc.scalar.activation(lv[:], ps_sq[:], AF.Ln)
                nc.scalar.activation(sv[:], lv[:], AF.Exp, scale=0.5)
                if it == 0:
                    nc.vector.tensor_scalar(uv[:], ps_sq[:],
                                            1.0 / (an * an), 1.0,
                                            op0=ALU.mult, op1=ALU.add)
                else:
                    nc.vector.tensor_scalar_add(uv[:], ps_sq[:], 1.0)
                nc.vector.reciprocal(rv[:], uv[:])
                nc.vector.tensor_tensor(fv[:], sv[:], rv[:], op=ALU.mult)
                ps_fr = pp.tile([128, NPAIR], F32, tag="sm", bufs=2,
                                name=f"ps_fr_{it}")
                rep = inv_an2 if it == 0 else one_row
                nc.tensor.matmul(ps_fr[:], rep[:], fv[:], start=True, stop=True)
                nc.vector.tensor_tensor(outv[:], vote[:], ps_fr[:], op=ALU.mult)

            def deltas(it):
                ov = outv[:]
                ov_b = bass.AP(ov.tensor, ov.offset,
                               [ov.ap[0], [0, an], [1, NPAIR]])
                nc.vector.tensor_tensor(
                    scr288[:].rearrange("p (n pair) -> p n pair", n=an),
                    ps_pr[:].rearrange("p (n pair) -> p n pair", n=an),
                    ov_b, op=ALU.mult)
                nc.tensor.matmul(ps_dl[:], ones128[:], scr288[:],
                                 start=(it == 0), stop=True)

            def softmax_probs(it):
                dl = ps_dl[:]
                dl_pn = bass.AP(dl.tensor, dl.offset,
                                [dl.ap[0], [1, NPAIR], [NPAIR, an]])
                nc.vector.tensor_reduce(mx[:], dl_pn,
                                        axis=mybir.AxisListType.X, op=ALU.max)
                mxa = mx[:]
                mx_b = bass.AP(mxa.tensor, mxa.offset,
                               [mxa.ap[0], [0, an], [1, NPAIR]])
                nc.vector.tensor_tensor(
                    l2[:].rearrange("p (n pair) -> p n pair", n=an),
                    dl.rearrange("p (n pair) -> p n pair", n=an),
                    mx_b, op=ALU.subtract)
                nc.scalar.activation(ex[:], l2[:], AF.Exp)
                exa = ex[:]
                ex_pn = bass.AP(exa.tensor, exa.offset,
                                [exa.ap[0], [1, NPAIR], [NPAIR, an]])
                nc.vector.tensor_reduce(es[:], ex_pn,
                                        axis=mybir.AxisListType.X, op=ALU.add)
                nc.vector.reciprocal(ers[:], es[:])
                ersa = ers[:]
                ers_b = bass.AP(ersa.tensor, ersa.offset,
                                [ersa.ap[0], [0, an], [1, NPAIR]])
                nc.vector.tensor_tensor(
                    probs16[:].rearrange("p (n pair) -> p n pair", n=an),
                    ex[:].rearrange("p (n pair) -> p n pair", n=an),
                    ers_b, op=ALU.mult)

            pb_sb = rp.tile([128, ANP], F32)

            def vote_from_probs(it):
                ps_pb = pp.tile([128, ANP], F32, tag="sm", bufs=2,
                                name=f"ps_pb_{it}")
                nc.tensor.matmul(ps_pb[:], ones1_16[:], probs16[:],
                                 start=True, stop=True)
                nc.scalar.copy(pb_sb[:], ps_pb[:])
                nc.vector.tensor_tensor(scr288[:], ps_pr[:], pb_sb[:],
                                        op=ALU.mult)
                sc = scr288[:]
                sc_pn = bass.AP(sc.tensor, sc.offset,
                                [sc.ap[0], [1, NPAIR], [NPAIR, an]])
                nc.vector.tensor_reduce(vote[:], sc_pn,
                                        axis=mybir.AxisListType.X, op=ALU.add)

            # iter 0: probs uniform = 1/an (vote holds an*true vote)
            nc.vector.tensor_reduce(vote[:], pr_pn,
                                    axis=mybir.AxisListType.X, op=ALU.add)
            squash_vote(0)
            deltas(0)
            # iter 1
            softmax_probs(1)
            vote_from_probs(1)
            squash_vote(1)
            deltas(1)
            # iter 2 (final vote, unsquashed)
            softmax_probs(2)
            vote_from_probs(2)

            if dbg:
                nc.sync.dma_start(d_dvote[:], vote[:])

            # ---------------- phase 5: vote -> vt2 (fp8, DR layout) ------
            ps_tr = pp.tile([NPAIR, S], F32, tag="sm", bufs=2, name="ps_tr")
            nc.tensor.matmul(ps_tr[:], vote[:], ident_sb[:],
                             is_transpose=True)
            votef8 = rp.tile([NPAIR, S], F8)
            nc.scalar.activation(votef8[:], ps_tr[:], AF.Copy, scale=4.0)
            nc.sync.dma_start(
                d_vcb.ap().rearrange("(p s) -> p s", p=NPAIR), votef8[:])
            # vt2[c-row, i*S+s] = vote_flat[384*i + 3*s + c]  (row-major
            # reshape of [C*B, S] votes mixes pair rows within a token!)
            vflat = d_vcb.ap()
            for c in range(C):
                p_row, ksub = c % 2, c // 2
                src = bass.AP(vflat.tensor, vflat.offset + c,
                              [[3 * S, BL], [3, S]])
                nc.sync.dma_start(
                    vt2_sb[p_row:p_row + 1, ksub * TOK:(ksub + 1) * TOK]
                    .rearrange("p (i s) -> p i s", i=BL),
                    src)

            # ---------------- phase 6: fc1 (+m1 term) --------------------
            f1 = fc1w_sb[:]
            x8 = xt8_sb[:]
            m1a = m1_sb[:]
            v2a = vt2_sb[:]
            for ak in range(AK):
                for half in range(2):
                    bi = ak * 2 + half
                    ps_z = pp.tile([128, 1024], F32, tag="z1", bufs=2,
                                   name=f"ps_z_{bi}")
                    for tb4 in range(4):
                        tb = half * 4 + tb4
                        dst = ps_z[:, tb4 * 256:(tb4 + 1) * 256]
                        for hp in range(HKP):
                            lhsT = bass.AP(f1.tensor,
                                           f1.offset + hp * 2 * A + ak * 128,
                                           [f1.ap[0], [A, 2], [1, 128]])
                            rhs = bass.AP(x8.tensor,
                                          x8.offset + hp * 2 * TOK + tb * 256,
                                          [x8.ap[0], [TOK, 2], [1, 256]])
                            nc.tensor.matmul(dst, lhsT, rhs, perf_mode=DR,
                                             start=(hp == 0), stop=False)
                        lhsT = bass.AP(m1a.tensor, m1a.offset + ak * 128,
                                       [m1a.ap[0], [A, 2], [1, 128]])
                        rhs = bass.AP(v2a.tensor, v2a.offset + tb * 256,
                                      [v2a.ap[0], [TOK, 2], [1, 256]])
                        nc.tensor.matmul(dst, lhsT, rhs, perf_mode=DR,
                                         start=False, stop=True)
                    zdst = z1_sb[:, ak * TOK + half * 1024:
                                 ak * TOK + (half + 1) * 1024]
                    if bi % 2 == 0:
                        nc.scalar.activation(zdst, ps_z[:], AF.Relu,
                                             bias=b1_sb[:, ak:ak + 1])
                    else:
                        nc.vector.tensor_scalar(zdst, ps_z[:],
                                                b1_sb[:, ak:ak + 1], 0.0,
                                                op0=ALU.add, op1=ALU.max)

            if dbg:
                nc.sync.dma_start(d_dz1[:], z1_sb[:])

            # ---------------- phase 7: fc2 + residual + out --------------
            f2 = fc2w_sb[:]
            z1a = z1_sb[:]
            for hk in range(HK):
                for half in range(2):
                    bi = hk * 2 + half
                    ps_f = pp.tile([128, 1024], F32, tag="z1", bufs=2,
                                   name=f"ps_f_{bi}")
                    for tb4 in range(4):
                        tb = half * 4 + tb4
                        dst = ps_f[:, tb4 * 256:(tb4 + 1) * 256]
                        for ap_ in range(AKP):
                            lhsT = bass.AP(f2.tensor,
                                           f2.offset + ap_ * 2 * H + hk * 128,
                                           [f2.ap[0], [H, 2], [1, 128]])
                            rhs = bass.AP(z1a.tensor,
                                          z1a.offset + 2 * ap_ * TOK + tb * 256,
                                          [z1a.ap[0], [TOK, 2], [1, 256]])
                            nc.tensor.matmul(dst, lhsT, rhs, perf_mode=DR,
                                             start=(ap_ == 0),
                                             stop=(ap_ == AKP - 1))
                    xs = x16_sb[:, hk * TOK + half * 1024:
                                hk * TOK + (half + 1) * 1024]
                    ot = sp.tile([128, 1024], F16, tag="ot", bufs=2,
                                 name=f"ot_{bi}")
                    h16 = sp.tile([128, 1024], F16, tag="h16", bufs=2,
                                  name=f"h16_{bi}")
                    nc.scalar.activation(h16[:], ps_f[:], AF.Relu,
                                         bias=b2a_sb[:, hk:hk + 1],
                                         scale=1.0 / 256.0)
                    if bi % 2 == 0:
                        nc.vector.tensor_tensor(ot[:], h16[:], xs, op=ALU.add)
                    else:
                        nc.gpsimd.tensor_tensor(ot[:], h16[:], xs, op=ALU.add)
                    nc.sync.dma_start(
                        d_out.ap()[hk][:, half * 1024:(half + 1) * 1024],
                        ot[:])

    nc.compile()
    return nc


# ---------------------------------------------------------------------------
# host marshaling
# ---------------------------------------------------------------------------

def _prep_core_inputs(k, x, shared, act_n):
    an = act_n
    own = np.array([(48 * k + 43 * i) % B for i in range(BL)])
    sem_ex = np.array([(16 * k + j) % B for j in range(NSEM)])

    # xtsem[hk, p, j*S+s] = x[sem_ex[j], s, hk*128+p]  (fp16)
    xs = np.transpose(x[sem_ex], (2, 0, 1)).reshape(H, NSEM_S)
    xtsem = _f16(xs).reshape(HK, 128, NSEM_S)

    # masks[p_any, g*NPAIR + pair] = 1 if rw group of pair == g
    masks = np.zeros((3, NPAIR), np.float32)
    for i in range(BL):
        for u in range(C):
            t = 3 * int(own[i]) + u
            masks[t // B, 3 * i + u] = 1.0
    masks_rep = _f16(np.broadcast_to(masks.reshape(1, 3 * NPAIR), (128, 3 * NPAIR)))

    # xt8[hp, p, ksub*TOK + e*S+s] = x[own[e], s, (2hp+ksub)*128+p]  (fp8)
    xo = np.transpose(x[own], (2, 0, 1)).reshape(H, TOK)  # [h, tok]
    xt8 = _f8(xo.reshape(HKP, 2, 128, TOK).transpose(0, 2, 1, 3)
              .reshape(HKP, 128, 2 * TOK))

    # x16[hk, p, tok]
    x16 = _f16(xo).reshape(HK, 128, TOK)

    return {
        "xtsem": xtsem,
        "masks": masks_rep,
        "xt8": xt8,
        "x16": x16,
        **{n: shared[n] for n in ("sw", "semb", "rw", "ident", "m1", "b1",
                                  "b2a", "b2b", "fc1w", "fc2w")},
    }


_CACHE = {}


def _make_shared(inputs):
    fc1_w = np.asarray(inputs["fc1_w"], np.float32)
    fc1_b = np.asarray(inputs["fc1_b"], np.float32)
    fc2_w = np.asarray(inputs["fc2_w"], np.float32)
    fc2_b = np.asarray(inputs["fc2_b"], np.float32)
    efc1 = np.asarray(inputs["efc1"], np.float32)
    efc2 = np.asarray(inputs["efc2"], np.float32)
    sem_w = np.asarray(inputs["sem_w"], np.float32)
    sem_b = np.asarray(inputs["sem_b"], np.float32)
    route_weights = np.asarray(inputs["route_weights"], np.float32)
    larger_w = np.asarray(inputs["larger_w"], np.float32)
    larger_b = np.asarray(inputs["larger_b"], np.float32)
    elarger = np.asarray(inputs["elarger"], np.float32)
    t = int(np.asarray(inputs["t"]))
    sf = np.float32(int(np.asarray(inputs["s"])))
    an = t + 1

    gfc1 = _sigmoid_f32(sf * efc1[t])
    gfc2 = _sigmoid_f32(sf * efc2[t])
    glarger = _sigmoid_f32(sf * elarger[t])

    lwg = larger_w * glarger[None, :]
    lb_eff = larger_b * glarger
    m1 = lwg @ fc1_w                                  # [C, A]
    b1 = (fc1_b + lb_eff @ fc1_w).astype(np.float32)  # [A]
    w2 = fc2_w * gfc1[:, None] * gfc2[None, :]
    b2 = (fc2_b * gfc2).astype(np.float32)            # [H]

    # m1 packed for DoubleRow with K=4 (rows: c=2*ksub+p, row3 zero), x4 scale
    m14 = np.zeros((2, 2, A), np.float32)
    for c in range(C):
        m14[c % 2, c // 2] = 4.0 * m1[c]

    # fc1w[hp, p, ksub*A + a] = 16*fc1_w[(2hp+ksub)*128+p, a]
    fc1w8 = _f8((16.0 * fc1_w).reshape(HKP, 2, 128, A).transpose(0, 2, 1, 3)
                .reshape(HKP, 128, 2 * A))
    # fc2w[ap, p, ksub*H + h] = 16*w2[(2ap+ksub)*128+p, h]
    fc2w8 = _f8((16.0 * w2).reshape(AKP, 2, 128, H).transpose(0, 2, 1, 3)
                .reshape(AKP, 128, 2 * H))

    # rw[g, s, (n, c, d)] = route_weights[g, n, s*C+c -> (s, c), d]  fp16
    rw4 = route_weights.reshape(C, N, S, C, S)
    rw16 = np.stack([
        _f16(np.transpose(rw4[g, :an], (1, 0, 2, 3)).reshape(S, an * C * S))
        for g in range(C)])

    return {
        "sw": _f16(np.transpose(sem_w, (1, 0, 2)).reshape(H, NC30)
                   ).reshape(HK, 128, NC30),
        "semb": _f16(sem_b.reshape(1, NC30)),
        "rw": rw16,
        "ident": np.ascontiguousarray(np.eye(128, dtype=np.float32)),
        "m1": _f8(m14.reshape(2, 2 * A)),
        "b1": np.ascontiguousarray((16.0 * b1).reshape(AK, 128).T),
        "b2a": np.ascontiguousarray(b2.reshape(HK, 128).T),
        "b2b": np.ascontiguousarray((256.0 * b2).reshape(HK, 128).T),
        "fc1w": fc1w8,
        "fc2w": fc2w8,
    }


def kernel(**inputs):
    x = np.asarray(inputs["x"], np.float32)
    t = int(np.asarray(inputs["t"]))
    an = t + 1
    shared = _make_shared(inputs)

    if an not in _CACHE:
        _CACHE[an] = _build_program(an)
    nc = _CACHE[an]

    in_maps = [_prep_core_inputs(k, x, shared, an) for k in range(NCORES)]
    res = bass_utils.run_bass_kernel_spmd(nc, in_maps, core_ids=list(range(NCORES)))
    out = np.empty((B, S, H), np.float32)
    for k in range(NCORES):
        own = [(48 * k + 43 * i) % B for i in range(BL)]
        # out dram [HK, 128, TOK] f16: [hk, p, e*S+s] -> x[own[e], s, hk*128+p]
        oc = np.asarray(res.results[k]["out"], np.float32)
        oc = oc.reshape(HK, 128, BL, S).transpose(2, 3, 0, 1).reshape(BL, S, H)
        out[own] = oc
    return out


# revision 34
# speedup vs baseline: 2.5570x; 1.1697x over previous
"""Trainium2 Bass kernel for nn_BertAdapterCapsuleMask (v2).

Strategy (8 NeuronCores, SPMD, data-parallel over batch):

Core k owns examples b_i = (48k + 43i) mod 128 (i<16).  Their routing
pairs t = 3b+u are exactly vote rows [48k,48k+48) and reference sem
examples [16k,16k+18) (consecutive) — zero cross-core traffic.

Key speed levers vs the v1 kernel (297us):
  * adapter GEMMs (fc1/fc2/m1-term) run in fp8-e4m3 DoubleRow mode
    (2 k-subtiles per matmul, 0.5 cyc/out-column = 4x bf16 throughput).
    Weights are pre-scaled by 16 (and vt/m1 by 4) to dodge e4m3's
    denormal floor; scales are unwound in the psum->sbuf drains.
  * single-pass fc1: x-part, capsule (m1) part and bias accumulate in
    PSUM; one fused relu drain emits z1=16*relu(.) straight to fp8.
    This kills v1's z1p roundtrip (72us Act + 42us DVE).
  * sem/priors matmuls in fp16; priors produced directly in a
    [d=128, (n,pair)] psum so the 3-iter routing loop is ~30 small
    free-dim-billed vector ops + tiny helper matmuls (column reduction
    and partition replication via ones-matmuls).
  * drains/adds round-robin over Act/Pool/DVE; residual x and output
    travel as fp16 ([h, token] layout, transposed back on host).

Numerics (validated host-side): rel-err ~1.4e-2 vs the 2e-2 gate;
sem/priors fp16, routing fp32, adapters fp8, output fp16.
"""

import numpy as np
import ml_dtypes

import concourse.bass as bass
import concourse.bacc as bacc
import concourse.mybir as mybir
import concourse.tile as tile
from concourse import bass_utils

F8NP = ml_dtypes.float8_e4m3
F16NP = np.float16
F32 = mybir.dt.float32
F16 = mybir.dt.float16
F8 = mybir.dt.float8e4
AF = mybir.ActivationFunctionType
ALU = mybir.AluOpType
DR = mybir.MatmulPerfMode.DoubleRow

B, S, H, A, C, N = 128, 128, 768, 2048, 3, 10
NCORES = 8
BL = B // NCORES            # 16 own examples / core
NPAIR = 3 * BL              # 48 routing pairs / core
NSEM = 18                   # sem examples / core
TOK = BL * S                # 2048 own tokens / core
HK = H // 128               # 6
HKP = HK // 2               # 3 h double-chunks
AK = A // 128               # 16
AKP = AK // 2               # 8 a double-chunks
NC30 = N * C                # 30 sem cols / slot
NSEM_S = NSEM * S           # 2304


def _sigmoid_f32(z):
    z = np.asarray(z, np.float32)
    out = np.empty_like(z)
    pos = z >= 0
    out[pos] = 1.0 / (1.0 + np.exp(-z[pos], dtype=np.float32))
    ez = np.exp(z[~pos], dtype=np.float32)
    out[~pos] = ez / (1.0 + ez)
    return out.astype(np.float32)


def _f8(x):
    return np.ascontiguousarray(np.asarray(x, np.float32).astype(F8NP))


def _f16(x):
    return np.ascontiguousarray(np.asarray(x, np.float32).astype(F16NP))


# ---------------------------------------------------------------------------
# device program
# ---------------------------------------------------------------------------

def _build_program(act_n, dbg=False):
    an = act_n
    ANP = an * NPAIR            # routing free size (n-major, pair)
    nc = bacc.Bacc("TRN2", target_bir_lowering=False, debug=False,
                   num_devices=NCORES)

    d_sw = nc.dram_tensor("sw", [HK, 128, NC30], F16, kind="ExternalInput")
    d_semb = nc.dram_tensor("semb", [1, NC30], F16, kind="ExternalInput")
    # xtsem grouped: [grp, hk, p, jj*S+s] for slots j = 6*grp + jj
    d_xtsem = nc.dram_tensor("xtsem", [3, HK, 128, 6 * S], F16,
                             kind="ExternalInput")
    d_masks = nc.dram_tensor("masks", [128, 3 * NPAIR], F16, kind="ExternalInput")
    d_rw = nc.dram_tensor("rw", [C, 128, an * C * S], F16, kind="ExternalInput")
    d_perm = nc.dram_tensor("perm", [C, 128, 128], F32, kind="ExternalInput")
    d_m1 = nc.dram_tensor("m1", [2, 2 * A], F8, kind="ExternalInput")
    d_b1 = nc.dram_tensor("b1", [128, AK], F32, kind="ExternalInput")
    d_b2a = nc.dram_tensor("b2a", [128, HK], F32, kind="ExternalInput")
    d_b2b = nc.dram_tensor("b2b", [128, HK], F32, kind="ExternalInput")
    d_xt8 = nc.dram_tensor("xt8", [HKP, 128, 2 * TOK], F8, kind="ExternalInput")
    d_fc1w = nc.dram_tensor("fc1w", [HKP, 128, 2 * A], F8, kind="ExternalInput")
    d_x16 = nc.dram_tensor("x16", [HK, 128, TOK], F16, kind="ExternalInput")
    d_fc2w = nc.dram_tensor("fc2w", [AKP, 128, 2 * H], F8, kind="ExternalInput")
    d_out = nc.dram_tensor("out", [HK, 128, TOK], F16, kind="ExternalOutput")
    if dbg:
        d_dsem = nc.dram_tensor("dsem", [128, NSEM * NC30], F32,
                                kind="ExternalOutput")
        d_dsp = nc.dram_tensor("dsp", [128, NPAIR * an * C], F16,
                               kind="ExternalOutput")
        d_dpr = nc.dram_tensor("dpr", [128, ANP], F32, kind="ExternalOutput")
        d_dvote = nc.dram_tensor("dvote", [128, NPAIR], F32,
                                 kind="ExternalOutput")
        d_dz1 = nc.dram_tensor("dz1", [128, AK * TOK], F8,
                               kind="ExternalOutput")

    with tile.TileContext(nc) as tc:
        with (
            tc.tile_pool(name="w", bufs=1) as wp,
            tc.tile_pool(name="rt", bufs=1) as rp,
            tc.tile_pool(name="st", bufs=2) as sp,
            tc.tile_pool(name="ps", bufs=1, space="PSUM") as pp,
        ):
            # load the ln/exp table once: covers Ln/Exp/Relu/Copy for the
            # whole program (avoids per-switch 1.3us table loads)
            from concourse.hw_specs import get_activation_tables
            _tables = list(get_activation_tables(nc.m.arch).keys())
            _nle = _tables.index("natural_log_exp_and_others")
            nc.scalar.add_instruction(mybir.InstLoadActFuncSet(
                name=nc.get_next_instruction_name(), ins=[], outs=[],
                act_func_set_id=_nle))

            # ---------------- DMAs: ordered by first use -----------------
            sw_sb = wp.tile([128, HK * NC30], F16)
            nc.sync.dma_start(
                sw_sb[:].rearrange("p (hk c) -> p hk c", hk=HK),
                d_sw.ap().rearrange("hk p c -> p hk c"))
            semb_sb = wp.tile([1, NC30], F16)
            nc.sync.dma_start(semb_sb[:], d_semb[:])
            # xtsem_sb layout: (grp, hk, jj*S+s)
            xtsem_sb = wp.tile([128, 3 * HK * 6 * S], F16, tag="xts")
            for grp in range(3):
                nc.sync.dma_start(
                    xtsem_sb[:, grp * HK * 6 * S:(grp + 1) * HK * 6 * S]
                    .rearrange("p (hk c) -> p hk c", hk=HK),
                    d_xtsem.ap()[grp].rearrange("hk p c -> p hk c"))
            masks_sb = wp.tile([128, 3 * NPAIR], F16)
            nc.sync.dma_start(masks_sb[:], d_masks[:])
            rw_sb = wp.tile([128, C * an * C * S], F16, tag="rw")
            nc.sync.dma_start(
                rw_sb[:].rearrange("p (g c) -> p g c", g=C),
                d_rw.ap().rearrange("g p c -> p g c"))
            perm_sb = wp.tile([128, C * 128], F32)
            nc.sync.dma_start(
                perm_sb[:].rearrange("p (g c) -> p g c", g=C),
                d_perm.ap().rearrange("g p c -> p g c"))
            m1_sb = wp.tile([2, 2 * A], F8)
            nc.sync.dma_start(m1_sb[:], d_m1[:])
            b1_sb = wp.tile([128, AK], F32)
            nc.sync.dma_start(b1_sb[:], d_b1[:])
            b2a_sb = wp.tile([128, HK], F32)
            nc.sync.dma_start(b2a_sb[:], d_b2a[:])
            b2b_sb = wp.tile([128, HK], F32)
            nc.sync.dma_start(b2b_sb[:], d_b2b[:])
            xt8_sb = wp.tile([128, HKP * 2 * TOK], F8, tag="xt8")
            nc.sync.dma_start(
                xt8_sb[:].rearrange("p (hp c) -> p hp c", hp=HKP),
                d_xt8.ap().rearrange("hp p c -> p hp c"))
            fc1w_sb = wp.tile([128, HKP * 2 * A], F8, tag="fc1w")
            nc.sync.dma_start(
                fc1w_sb[:].rearrange("p (hp c) -> p hp c", hp=HKP),
                d_fc1w.ap().rearrange("hp p c -> p hp c"))
            x16_sb = wp.tile([128, HK * TOK], F16, tag="x16")
            nc.sync.dma_start(
                x16_sb[:].rearrange("p (hk c) -> p hk c", hk=HK),
                d_x16.ap().rearrange("hk p c -> p hk c"))
            fc2w_sb = wp.tile([128, AKP * 2 * H], F8, tag="fc2w")
            nc.sync.dma_start(
                fc2w_sb[:].rearrange("p (ap c) -> p ap c", ap=AKP),
                d_fc2w.ap().rearrange("ap p c -> p ap c"))

            # constants
            ones1_16 = wp.tile([1, 128], F16)
            nc.gpsimd.memset(ones1_16[:], 1.0)
            ones128 = wp.tile([128, 1], F32)
            nc.gpsimd.memset(ones128[:], 1.0)
            inv_an2 = wp.tile([1, 128], F32)
            nc.gpsimd.memset(inv_an2[:], 1.0 / (an * an))
            one_row = wp.tile([1, 128], F32)
            nc.gpsimd.memset(one_row[:], 1.0)
            vt2_sb = wp.tile([2, 2 * TOK], F8)
            nc.gpsimd.memset(vt2_sb[:], 0.0)

            # z1 lives across fc1->fc2
            z1_sb = wp.tile([128, AK * TOK], F8, tag="z1")

            # ---------------- phase 1: sem ([s,30] per slot) -------------
            sem_own = rp.tile([128, NSEM * NC30], F32)
            for grp in range(3):
                ps = pp.tile([128, 6 * NC30], F32, tag="sm", bufs=2,
                             name=f"ps_sem_{grp}")
                for jj in range(6):
                    j = grp * 6 + jj
                    dst = ps[:, jj * NC30:(jj + 1) * NC30]
                    base = grp * HK * 6 * S
                    for hk in range(HK):
                        nc.tensor.matmul(
                            dst,
                            xtsem_sb[:, base + hk * 6 * S + jj * S:
                                     base + hk * 6 * S + (jj + 1) * S],
                            sw_sb[:, hk * NC30:(hk + 1) * NC30],
                            start=(hk == 0), stop=False)
                    nc.tensor.matmul(dst, ones1_16[:], semb_sb[:],
                                     start=False, stop=True)
                nc.scalar.copy(
                    sem_own[:, grp * 6 * NC30:(grp + 1) * 6 * NC30], ps[:])

            # ---------------- phase 2: squash + sem_pair -----------------
            sem2 = rp.tile([128, NSEM * NC30], F32)
            nc.vector.tensor_tensor(sem2[:], sem_own[:], sem_own[:], op=ALU.mult)
            sqt = rp.tile([128, NSEM * C], F32)
            nc.vector.tensor_reduce(
                sqt[:].rearrange("p (slot cc) -> p slot cc", cc=C),
                sem2[:].rearrange("p (slot n cc) -> p slot cc n", n=N, cc=C),
                axis=mybir.AxisListType.X, op=ALU.add)
            lnq = rp.tile([128, NSEM * C], F32)
            nc.scalar.activation(lnq[:], sqt[:], AF.Ln)
            sqq = rp.tile([128, NSEM * C], F32)
            nc.scalar.activation(sqq[:], lnq[:], AF.Exp, scale=0.5)
            up = rp.tile([128, NSEM * C], F32)
            nc.vector.tensor_scalar_add(up[:], sqt[:], 1.0)
            ru = rp.tile([128, NSEM * C], F32)
            nc.vector.reciprocal(ru[:], up[:])
            fq = rp.tile([128, NSEM * C], F32)
            nc.vector.tensor_tensor(fq[:], sqq[:], ru[:], op=ALU.mult)
            # expand f to (slot, n, c) so the pair gather stays 3-dim
            fq18 = rp.tile([128, NSEM * an * C], F32)
            fqa = fq[:]
            fq_b = bass.AP(fqa.tensor, fqa.offset,
                           [fqa.ap[0], [C, NSEM], [0, an], [1, C]])
            nc.vector.tensor_copy(
                fq18[:].rearrange("p (slot n c) -> p slot n c", n=an, c=C),
                fq_b)

            # sem_pair[p, (pair, n, c)] = sem_own[s, (slot(pair), n, c)] * fq
            spair = rp.tile([128, NPAIR * an * C], F16)
            so = sem_own[:]
            gather = bass.AP(so.tensor, so.offset,
                             [so.ap[0], [NC30, BL], [NC30, C], [1, an * C]])
            f18 = fq18[:]
            fgather = bass.AP(f18.tensor, f18.offset,
                              [f18.ap[0], [an * C, BL], [an * C, C], [1, an * C]])
            nc.vector.tensor_tensor(
                spair[:].rearrange("p (i u nc) -> p i u nc", i=BL, u=C),
                gather, fgather, op=ALU.mult)

            # masked copies (one per rw group g); one rides on Pool
            spg = rp.tile([128, 3 * NPAIR * an * C], F16)
            ms = masks_sb[:]
            for g in range(C):
                mask_b = bass.AP(ms.tensor, ms.offset + g * NPAIR,
                                 [ms.ap[0], [1, NPAIR], [0, an * C]])
                eng = nc.gpsimd if g == 2 else nc.vector
                eng.tensor_tensor(
                    spg[:, g * NPAIR * an * C:(g + 1) * NPAIR * an * C]
                    .rearrange("p (pair nc) -> p pair nc", nc=an * C),
                    spair[:].rearrange("p (pair nc) -> p pair nc", nc=an * C),
                    mask_b, op=ALU.mult)

            if dbg:
                nc.sync.dma_start(d_dsem[:], sem_own[:])
                nc.sync.dma_start(d_dsp[:], spair[:])

            # ---------------- phase 3: priors [d, (n, pair)] -------------
            ps_pr = pp.tile([128, ANP], F32, tag="pr", name="ps_pr")
            spg_ap = spg[:]
            for n in range(an):
                first = True
                for g in range(C):
                    for cc in range(C):
                        mov = bass.AP(
                            spg_ap.tensor,
                            spg_ap.offset + g * NPAIR * an * C + n * C + cc,
                            [spg_ap.ap[0], [an * C, NPAIR]])
                        nc.tensor.matmul(
                            ps_pr[:, n * NPAIR:(n + 1) * NPAIR],
                            rw_sb[:, g * an * C * S + n * C * S + cc * S:
                                  g * an * C * S + n * C * S + (cc + 1) * S],
                            mov,
                            start=first, stop=(g == C - 1 and cc == C - 1))
                        first = False

            # priors to SBUF once: routing reads never touch two PSUMs
            pr_sb = rp.tile([128, ANP], F32)
            nc.scalar.copy(pr_sb[:], ps_pr[:])
            if dbg:
                nc.sync.dma_start(d_dpr[:], pr_sb[:])

            # ---------------- phase 4: routing ---------------------------
            # priors view [p partitions(d), pair, n] with n innermost
            pr = pr_sb[:]
            pr_pn = bass.AP(pr.tensor, pr.offset,
                            [pr.ap[0], [1, NPAIR], [NPAIR, an]])

            vote = rp.tile([128, NPAIR], F32)
            scr48 = rp.tile([128, NPAIR], F32)
            outv = rp.tile([128, NPAIR], F32)
            scr288 = rp.tile([128, ANP], F32)
            lv = rp.tile([1, NPAIR], F32)
            sv = rp.tile([1, NPAIR], F32)
            uv = rp.tile([1, NPAIR], F32)
            rv = rp.tile([1, NPAIR], F32)
            fv = rp.tile([1, NPAIR], F32)
            mx = rp.tile([1, NPAIR], F32)
            l2 = rp.tile([1, ANP], F32)
            ex = rp.tile([1, ANP], F32)
            es = rp.tile([1, NPAIR], F32)
            ers = rp.tile([1, NPAIR], F32)
            probs16 = rp.tile([1, ANP], F16)

            ps_dl = pp.tile([1, ANP], F32, tag="dl", name="ps_dl")

            def squash_vote(it):
                # f = sqrt(sq)/(1+sq); it0 vote is an*true -> rescale consts
                nc.vector.tensor_tensor(scr48[:], vote[:], vote[:], op=ALU.mult)
                ps_sq = pp.tile([1, NPAIR], F32, tag="sm", bufs=2,
                                name=f"ps_sq_{it}")
                nc.tensor.matmul(ps_sq[:], ones128[:], scr48[:],
                                 start=True, stop=True)
                nc.scalar.activation(lv[:], ps_sq[:], AF.Ln)
                nc.scalar.activation(sv[:], lv[:], AF.Exp, scale=0.5)
                if it == 0:
                    nc.vector.tensor_scalar(uv[:], ps_sq[:],
                                            1.0 / (an * an), 1.0,
                                            op0=ALU.mult, op1=ALU.add)
                else:
                    nc.vector.tensor_scalar_add(uv[:], ps_sq[:], 1.0)
                nc.vector.reciprocal(rv[:], uv[:])
                nc.vector.tensor_tensor(fv[:], sv[:], rv[:], op=ALU.mult)
                ps_fr = pp.tile([128, NPAIR], F32, tag="sm", bufs=2,
                                name=f"ps_fr_{it}")
                rep = inv_an2 if it == 0 else one_row
                nc.tensor.matmul(ps_fr[:], rep[:], fv[:], start=True, stop=True)
                nc.vector.tensor_tensor(outv[:], vote[:], ps_fr[:], op=ALU.mult)

            def deltas(it):
                ov = outv[:]
                ov_b = bass.AP(ov.tensor, ov.offset,
                               [ov.ap[0], [0, an], [1, NPAIR]])
                nc.vector.tensor_tensor(
                    scr288[:].rearrange("p (n pair) -> p n pair", n=an),
                    pr_sb[:].rearrange("p (n pair) -> p n pair", n=an),
                    ov_b, op=ALU.mult)
                nc.tensor.matmul(ps_dl[:], ones128[:], scr288[:],
                                 start=(it == 0), stop=True)

            def softmax_probs(it):
                dl = ps_dl[:]
                dl_pn = bass.AP(dl.tensor, dl.offset,
                                [dl.ap[0], [1, NPAIR], [NPAIR, an]])
                nc.vector.tensor_reduce(mx[:], dl_pn,
                                        axis=mybir.AxisListType.X, op=ALU.max)
                mxa = mx[:]
                mx_b = bass.AP(mxa.tensor, mxa.offset,
                               [mxa.ap[0], [0, an], [1, NPAIR]])
                nc.vector.tensor_tensor(
                    l2[:].rearrange("p (n pair) -> p n pair", n=an),
                    dl.rearrange("p (n pair) -> p n pair", n=an),
                    mx_b, op=ALU.subtract)
                nc.scalar.activation(ex[:], l2[:], AF.Exp)
                exa = ex[:]
                ex_pn = bass.AP(exa.tensor, exa.offset,
                                [exa.ap[0], [1, NPAIR], [NPAIR, an]])
                nc.vector.tensor_reduce(es[:], ex_pn,
                                        axis=mybir.AxisListType.X, op=ALU.add)
                nc.vector.reciprocal(ers[:], es[:])
                ersa = ers[:]
                ers_b = bass.AP(ersa.tensor, ersa.offset,
                                [ersa.ap[0], [0, an], [1, NPAIR]])
                nc.vector.tensor_tensor(
                    probs16[:].rearrange("p (n pair) -> p n pair", n=an),
                    ex[:].rearrange("p (n pair) -> p n pair", n=an),
                    ers_b, op=ALU.mult)

            def vote_from_probs(it):
                ps_pb = pp.tile([128, ANP], F32, tag="sm", bufs=2,
                                name=f"ps_pb_{it}")
                nc.tensor.matmul(ps_pb[:], ones1_16[:], probs16[:],
                                 start=True, stop=True)
                nc.vector.tensor_tensor(scr288[:], pr_sb[:], ps_pb[:],
                                        op=ALU.mult)
                sc = scr288[:]
                sc_pn = bass.AP(sc.tensor, sc.offset,
                                [sc.ap[0], [1, NPAIR], [NPAIR, an]])
                nc.vector.tensor_reduce(vote[:], sc_pn,
                                        axis=mybir.AxisListType.X, op=ALU.add)

            # iter 0: probs uniform = 1/an (vote holds an*true vote)
            nc.vector.tensor_reduce(vote[:], pr_pn,
                                    axis=mybir.AxisListType.X, op=ALU.add)
            squash_vote(0)
            deltas(0)
            # iter 1
            softmax_probs(1)
            vote_from_probs(1)
            squash_vote(1)
            deltas(1)
            # iter 2 (final vote, unsquashed)
            softmax_probs(2)
            vote_from_probs(2)

            if dbg:
                nc.sync.dma_start(d_dvote[:], vote[:])

            # ---------------- phase 5: vote -> vt2 (fp8, DR layout) ------
            # Reference h_out[b,s,c] = vote_flat[384b + 3s + c]: pair row
            # 3i+u with u=(3s+c)//128, token s'=(3s+c)%128.  Per (c,u):
            # transpose vote columns {3i+u} through the permutation
            # s'=(3s+c)%128, take the u-segment of s, quantize into
            # vtI[i, c*128+s]; then 2 tiny partition-merging DMAs -> vt2.
            vtI = rp.tile([BL, C * S], F8)
            va = vote[:]
            for c in range(C):
                for u in range(C):
                    s0 = (128 * u - c + 2) // 3 if u > 0 else 0
                    s1 = (128 * (u + 1) - c + 2) // 3
                    s1 = min(s1, S)
                    if s1 <= s0:
                        continue
                    lhsT = bass.AP(va.tensor, va.offset + u,
                                   [va.ap[0], [3, BL]])
                    ps_t = pp.tile([BL, S], F32, tag="sm", bufs=2,
                                   name=f"ps_t_{c}_{u}")
                    nc.tensor.matmul(ps_t[:], lhsT,
                                     perm_sb[:, c * 128:(c + 1) * 128],
                                     is_transpose=True)
                    eng = nc.scalar if (c * C + u) % 2 == 0 else None
                    if eng is not None:
                        nc.scalar.activation(
                            vtI[:, c * S + s0:c * S + s1],
                            ps_t[:, s0:s1], AF.Copy, scale=4.0)
                    else:
                        nc.vector.tensor_scalar_mul(
                            vtI[:, c * S + s0:c * S + s1],
                            ps_t[:, s0:s1], 4.0)
            # vt2[p_row][ksub*TOK + i*128 + s] = vtI[i, (2*ksub+p_row)*128+s]
            vI = vtI[:]
            for p_row in range(2):
                for ksub in range(2):
                    cc = 2 * ksub + p_row
                    if cc >= C:
                        continue  # row (1,1) stays zero
                    src = bass.AP(vI.tensor, vI.offset + cc * S,
                                  [vI.ap[0], [1, S]])
                    nc.gpsimd.dma_start(
                        vt2_sb[p_row:p_row + 1,
                               ksub * TOK:(ksub + 1) * TOK]
                        .rearrange("p (i s) -> p i s", i=BL),
                        src)

            # ---------------- phase 6: fc1 (+m1 term) --------------------
            f1 = fc1w_sb[:]
            x8 = xt8_sb[:]
            m1a = m1_sb[:]
            v2a = vt2_sb[:]
            for ak in range(AK):
                for half in range(2):
                    bi = ak * 2 + half
                    ps_z = pp.tile([128, 1024], F32, tag="z1", bufs=2,
                                   name=f"ps_z_{bi}")
                    for tb4 in range(4):
                        tb = half * 4 + tb4
                        dst = ps_z[:, tb4 * 256:(tb4 + 1) * 256]
                        for hp in range(HKP):
                            lhsT = bass.AP(f1.tensor,
                                           f1.offset + hp * 2 * A + ak * 128,
                                           [f1.ap[0], [A, 2], [1, 128]])
                            rhs = bass.AP(x8.tensor,
                                          x8.offset + hp * 2 * TOK + tb * 256,
                                          [x8.ap[0], [TOK, 2], [1, 256]])
                            nc.tensor.matmul(dst, lhsT, rhs, perf_mode=DR,
                                             start=(hp == 0), stop=False)
                        lhsT = bass.AP(m1a.tensor, m1a.offset + ak * 128,
                                       [m1a.ap[0], [A, 2], [1, 128]])
                        rhs = bass.AP(v2a.tensor, v2a.offset + tb * 256,
                                      [v2a.ap[0], [TOK, 2], [1, 256]])
                        nc.tensor.matmul(dst, lhsT, rhs, perf_mode=DR,
                                         start=False, stop=True)
                    zdst = z1_sb[:, ak * TOK + half * 1024:
                                 ak * TOK + (half + 1) * 1024]
                    if bi % 2 == 0:
                        nc.scalar.activation(zdst, ps_z[:], AF.Relu,
                                             bias=b1_sb[:, ak:ak + 1])
                    else:
                        nc.vector.tensor_scalar(zdst, ps_z[:],
                                                b1_sb[:, ak:ak + 1], 0.0,
                                                op0=ALU.add, op1=ALU.max)

            if dbg:
                nc.sync.dma_start(d_dz1[:], z1_sb[:])

            # ---------------- phase 7: fc2 + residual + out --------------
            f2 = fc2w_sb[:]
            z1a = z1_sb[:]
            for hk in range(HK):
                for half in range(2):
                    bi = hk * 2 + half
                    ps_f = pp.tile([128, 1024], F32, tag="z1", bufs=2,
                                   name=f"ps_f_{bi}")
                    for tb4 in range(4):
                        tb = half * 4 + tb4
                        dst = ps_f[:, tb4 * 256:(tb4 + 1) * 256]
                        for ap_ in range(AKP):
                            lhsT = bass.AP(f2.tensor,
                                           f2.offset + ap_ * 2 * H + hk * 128,
                                           [f2.ap[0], [H, 2], [1, 128]])
                            rhs = bass.AP(z1a.tensor,
                                          z1a.offset + 2 * ap_ * TOK + tb * 256,
                                          [z1a.ap[0], [TOK, 2], [1, 256]])
                            nc.tensor.matmul(dst, lhsT, rhs, perf_mode=DR,
                                             start=(ap_ == 0),
                                             stop=(ap_ == AKP - 1))
                    xs = x16_sb[:, hk * TOK + half * 1024:
                                hk * TOK + (half + 1) * 1024]
                    ot = sp.tile([128, 1024], F16, tag="ot", bufs=2,
                                 name=f"ot_{bi}")
                    h16 = sp.tile([128, 1024], F16, tag="h16", bufs=2,
                                  name=f"h16_{bi}")
                    nc.scalar.activation(h16[:], ps_f[:], AF.Relu,
                                         bias=b2a_sb[:, hk:hk + 1],
                                         scale=1.0 / 256.0)
                    eng = nc.gpsimd if bi in (1, 3, 5, 7) else nc.vector
                    eng.tensor_tensor(ot[:], h16[:], xs, op=ALU.add)
                    nc.sync.dma_start(
                        d_out.ap()[hk][:, half * 1024:(half + 1) * 1024],
                        ot[:])

    nc.compile()
    return nc


# ---------------------------------------------------------------------------
# host marshaling
# ---------------------------------------------------------------------------

def _prep_core_inputs(k, x, shared, act_n):
    an = act_n
    own = np.array([(48 * k + 43 * i) % B for i in range(BL)])
    sem_ex = np.array([(16 * k + j) % B for j in range(NSEM)])

    # xtsem[grp, hk, p, jj*S+s] = x[sem_ex[6*grp+jj], s, hk*128+p]  (fp16)
    xs = _f16(np.transpose(x[sem_ex], (2, 0, 1)).reshape(H, NSEM_S))
    xtsem = np.ascontiguousarray(
        xs.reshape(HK, 128, 3, 6 * S).transpose(2, 0, 1, 3))

    # masks[p_any, g*NPAIR + pair] = 1 if rw group of pair == g
    masks = np.zeros((3, NPAIR), np.float32)
    for i in range(BL):
        for u in range(C):
            t = 3 * int(own[i]) + u
            masks[t // B, 3 * i + u] = 1.0
    masks_rep = _f16(np.broadcast_to(masks.reshape(1, 3 * NPAIR), (128, 3 * NPAIR)))

    # xt8[hp, p, ksub*TOK + e*S+s] = x[own[e], s, (2hp+ksub)*128+p]  (fp8)
    xo = np.transpose(x[own], (2, 0, 1)).reshape(H, TOK)  # [h, tok]
    xt8 = _f8(xo.reshape(HKP, 2, 128, TOK).transpose(0, 2, 1, 3)
              .reshape(HKP, 128, 2 * TOK))

    # x16[hk, p, tok]
    x16 = _f16(xo).reshape(HK, 128, TOK)

    return {
        "xtsem": xtsem,
        "masks": masks_rep,
        "xt8": xt8,
        "x16": x16,
        **{n: shared[n] for n in ("sw", "semb", "rw", "perm", "m1", "b1",
                                  "b2a", "b2b", "fc1w", "fc2w")},
    }


_CACHE = {}


def _make_shared(inputs):
    fc1_w = np.asarray(inputs["fc1_w"], np.float32)
    fc1_b = np.asarray(inputs["fc1_b"], np.float32)
    fc2_w = np.asarray(inputs["fc2_w"], np.float32)
    fc2_b = np.asarray(inputs["fc2_b"], np.float32)
    efc1 = np.asarray(inputs["efc1"], np.float32)
    efc2 = np.asarray(inputs["efc2"], np.float32)
    sem_w = np.asarray(inputs["sem_w"], np.float32)
    sem_b = np.asarray(inputs["sem_b"], np.float32)
    route_weights = np.asarray(inputs["route_weights"], np.float32)
    larger_w = np.asarray(inputs["larger_w"], np.float32)
    larger_b = np.asarray(inputs["larger_b"], np.float32)
    elarger = np.asarray(inputs["elarger"], np.float32)
    t = int(np.asarray(inputs["t"]))
    sf = np.float32(int(np.asarray(inputs["s"])))
    an = t + 1

    gfc1 = _sigmoid_f32(sf * efc1[t])
    gfc2 = _sigmoid_f32(sf * efc2[t])
    glarger = _sigmoid_f32(sf * elarger[t])

    lwg = larger_w * glarger[None, :]
    lb_eff = larger_b * glarger
    m1 = lwg @ fc1_w                                  # [C, A]
    b1 = (fc1_b + lb_eff @ fc1_w).astype(np.float32)  # [A]
    w2 = fc2_w * gfc1[:, None] * gfc2[None, :]
    b2 = (fc2_b * gfc2).astype(np.float32)            # [H]

    # m1 packed for DoubleRow with K=4 (rows: c=2*ksub+p, row3 zero), x4 scale
    m14 = np.zeros((2, 2, A), np.float32)
    for c in range(C):
        m14[c % 2, c // 2] = 4.0 * m1[c]

    # fc1w[hp, p, ksub*A + a] = 16*fc1_w[(2hp+ksub)*128+p, a]
    fc1w8 = _f8((16.0 * fc1_w).reshape(HKP, 2, 128, A).transpose(0, 2, 1, 3)
                .reshape(HKP, 128, 2 * A))
    # fc2w[ap, p, ksub*H + h] = 16*w2[(2ap+ksub)*128+p, h]
    fc2w8 = _f8((16.0 * w2).reshape(AKP, 2, 128, H).transpose(0, 2, 1, 3)
                .reshape(AKP, 128, 2 * H))

    # rw[g, s, (n, c, d)] = route_weights[g, n, s*C+c -> (s, c), d]  fp16
    rw4 = route_weights.reshape(C, N, S, C, S)
    rw16 = np.stack([
        _f16(np.transpose(rw4[g, :an], (1, 0, 2, 3)).reshape(S, an * C * S))
        for g in range(C)])

    # perm[c][s', s] = 1 iff s' == (3s+c) % 128
    perm = np.zeros((C, S, S), np.float32)
    for c in range(C):
        s = np.arange(S)
        perm[c, (3 * s + c) % S, s] = 1.0

    return {
        "sw": _f16(np.transpose(sem_w, (1, 0, 2)).reshape(H, NC30)
                   ).reshape(HK, 128, NC30),
        "semb": _f16(sem_b.reshape(1, NC30)),
        "rw": rw16,
        "perm": perm,
        "m1": _f8(m14.reshape(2, 2 * A)),
        "b1": np.ascontiguousarray((16.0 * b1).reshape(AK, 128).T),
        "b2a": np.ascontiguousarray(b2.reshape(HK, 128).T),
        "b2b": np.ascontiguousarray((256.0 * b2).reshape(HK, 128).T),
        "fc1w": fc1w8,
        "fc2w": fc2w8,
    }


def kernel(**inputs):
    x = np.asarray(inputs["x"], np.float32)
    t = int(np.asarray(inputs["t"]))
    an = t + 1
    shared = _make_shared(inputs)

    if an not in _CACHE:
        _CACHE[an] = _build_program(an)
    nc = _CACHE[an]

    in_maps = [_prep_core_inputs(k, x, shared, an) for k in range(NCORES)]
    res = bass_utils.run_bass_kernel_spmd(nc, in_maps, core_ids=list(range(NCORES)))
    out = np.empty((B, S, H), np.float32)
    for k in range(NCORES):
        own = [(48 * k + 43 * i) % B for i in range(BL)]
        # out dram [HK, 128, TOK] f16: [hk, p, e*S+s] -> x[own[e], s, hk*128+p]
        oc = np.asarray(res.results[k]["out"], np.float32)
        oc = oc.reshape(HK, 128, BL, S).transpose(2, 3, 0, 1).reshape(BL, S, H)
        out[own] = oc
    return out


# revision 46
# speedup vs baseline: 2.7275x; 1.0667x over previous
"""Trainium2 Bass kernel for nn_BertAdapterCapsuleMask (v2).

Strategy (8 NeuronCores, SPMD, data-parallel over batch):

Core k owns examples b_i = (48k + 43i) mod 128 (i<16).  Their routing
pairs t = 3b+u are exactly vote rows [48k,48k+48) and reference sem
examples [16k,16k+18) (consecutive) — zero cross-core traffic.

Key speed levers vs the v1 kernel (297us):
  * adapter GEMMs (fc1/fc2/m1-term) run in fp8-e4m3 DoubleRow mode
    (2 k-subtiles per matmul, 0.5 cyc/out-column = 4x bf16 throughput).
    Weights are pre-scaled by 16 (and vt/m1 by 4) to dodge e4m3's
    denormal floor; scales are unwound in the psum->sbuf drains.
  * single-pass fc1: x-part, capsule (m1) part and bias accumulate in
    PSUM; one fused relu drain emits z1=16*relu(.) straight to fp8.
    This kills v1's z1p roundtrip (72us Act + 42us DVE).
  * sem/priors matmuls in fp16; priors produced directly in a
    [d=128, (n,pair)] psum so the 3-iter routing loop is ~30 small
    free-dim-billed vector ops + tiny helper matmuls (column reduction
    and partition replication via ones-matmuls).
  * drains/adds round-robin over Act/Pool/DVE; residual x and output
    travel as fp16 ([h, token] layout, transposed back on host).

Numerics (validated host-side): rel-err ~1.4e-2 vs the 2e-2 gate;
sem/priors fp16, routing fp32, adapters fp8, output fp16.
"""

import numpy as np
import ml_dtypes

import concourse.bass as bass
import concourse.bacc as bacc
import concourse.mybir as mybir
import concourse.tile as tile
from concourse import bass_utils

F8NP = ml_dtypes.float8_e4m3
F16NP = np.float16
F32 = mybir.dt.float32
F16 = mybir.dt.float16
F8 = mybir.dt.float8e4
AF = mybir.ActivationFunctionType
ALU = mybir.AluOpType
DR = mybir.MatmulPerfMode.DoubleRow

B, S, H, A, C, N = 128, 128, 768, 2048, 3, 10
NCORES = 8
BL = B // NCORES            # 16 own examples / core
NPAIR = 3 * BL              # 48 routing pairs / core
NSEM = 18                   # sem examples / core
TOK = BL * S                # 2048 own tokens / core
HK = H // 128               # 6
HKP = HK // 2               # 3 h double-chunks
AK = A // 128               # 16
AKP = AK // 2               # 8 a double-chunks
NC30 = N * C                # 30 sem cols / slot
NSEM_S = NSEM * S           # 2304


def _sigmoid_f32(z):
    z = np.asarray(z, np.float32)
    out = np.empty_like(z)
    pos = z >= 0
    out[pos] = 1.0 / (1.0 + np.exp(-z[pos], dtype=np.float32))
    ez = np.exp(z[~pos], dtype=np.float32)
    out[~pos] = ez / (1.0 + ez)
    return out.astype(np.float32)


def _f8(x):
    return np.ascontiguousarray(np.asarray(x, np.float32).astype(F8NP))


def _f16(x):
    return np.ascontiguousarray(np.asarray(x, np.float32).astype(F16NP))


# ---------------------------------------------------------------------------
# device program
# ---------------------------------------------------------------------------

def _build_program(act_n, dbg=False):
    an = act_n
    ANP = an * NPAIR            # routing free size (n-major, pair)
    nc = bacc.Bacc("TRN2", target_bir_lowering=False, debug=False,
                   num_devices=NCORES)

    d_sw = nc.dram_tensor("sw", [HK, 128, NC30], F16, kind="ExternalInput")
    d_semb = nc.dram_tensor("semb", [1, NC30], F16, kind="ExternalInput")
    # xtsem grouped: [grp, hk, p, jj*S+s] for slots j = 6*grp + jj
    d_xtsem = nc.dram_tensor("xtsem", [3, HK, 128, 6 * S], F16,
                             kind="ExternalInput")
    d_masks = nc.dram_tensor("masks", [128, 3 * NPAIR], F16, kind="ExternalInput")
    d_rw = nc.dram_tensor("rw", [C, 128, an * C * S], F16, kind="ExternalInput")
    d_perm = nc.dram_tensor("perm", [C, 128, 128], F32, kind="ExternalInput")
    d_m1 = nc.dram_tensor("m1", [2, 2 * A], F8, kind="ExternalInput")
    d_b1 = nc.dram_tensor("b1", [128, AK], F32, kind="ExternalInput")
    d_b2a = nc.dram_tensor("b2a", [128, HK], F32, kind="ExternalInput")
    d_b2b = nc.dram_tensor("b2b", [128, HK], F32, kind="ExternalInput")
    d_xt8 = nc.dram_tensor("xt8", [HKP, 128, 2 * TOK], F8, kind="ExternalInput")
    d_fc1w = nc.dram_tensor("fc1w", [HKP, 128, 2 * A], F8, kind="ExternalInput")
    d_x16 = nc.dram_tensor("x16", [HK, 128, TOK], F16, kind="ExternalInput")
    d_fc2w = nc.dram_tensor("fc2w", [AKP, 128, 2 * H], F8, kind="ExternalInput")
    d_out = nc.dram_tensor("out", [HK, 128, TOK], F16, kind="ExternalOutput")
    if dbg:
        d_dsem = nc.dram_tensor("dsem", [128, NSEM * NC30], F32,
                                kind="ExternalOutput")
        d_dsp = nc.dram_tensor("dsp", [128, NPAIR * an * C], F16,
                               kind="ExternalOutput")
        d_dpr = nc.dram_tensor("dpr", [128, ANP], F32, kind="ExternalOutput")
        d_dvote = nc.dram_tensor("dvote", [128, NPAIR], F32,
                                 kind="ExternalOutput")
        d_dz1 = nc.dram_tensor("dz1", [128, AK * TOK], F8,
                               kind="ExternalOutput")

    with tile.TileContext(nc) as tc:
        with (
            tc.tile_pool(name="w", bufs=1) as wp,
            tc.tile_pool(name="rt", bufs=1) as rp,
            tc.tile_pool(name="st", bufs=2) as sp,
            tc.tile_pool(name="ps", bufs=1, space="PSUM") as pp,
        ):
            # load the ln/exp table once: covers Ln/Exp/Relu/Copy for the
            # whole program (avoids per-switch 1.3us table loads)
            from concourse.hw_specs import get_activation_tables
            _tables = list(get_activation_tables(nc.m.arch).keys())
            _nle = _tables.index("natural_log_exp_and_others")
            nc.scalar.add_instruction(mybir.InstLoadActFuncSet(
                name=nc.get_next_instruction_name(), ins=[], outs=[],
                act_func_set_id=_nle))

            # ---------------- DMAs: ordered by first use -----------------
            sw_sb = wp.tile([128, HK * NC30], F16)
            nc.sync.dma_start(
                sw_sb[:].rearrange("p (hk c) -> p hk c", hk=HK),
                d_sw.ap().rearrange("hk p c -> p hk c"))
            semb_sb = wp.tile([1, NC30], F16)
            nc.sync.dma_start(semb_sb[:], d_semb[:])
            # xtsem_sb layout: (grp, hk, jj*S+s)
            xtsem_sb = wp.tile([128, 3 * HK * 6 * S], F16, tag="xts")
            for grp in range(3):
                nc.sync.dma_start(
                    xtsem_sb[:, grp * HK * 6 * S:(grp + 1) * HK * 6 * S]
                    .rearrange("p (hk c) -> p hk c", hk=HK),
                    d_xtsem.ap()[grp].rearrange("hk p c -> p hk c"))
            masks_sb = wp.tile([128, 3 * NPAIR], F16)
            nc.sync.dma_start(masks_sb[:], d_masks[:])
            rw_sb = wp.tile([128, C * an * C * S], F16, tag="rw")
            nc.sync.dma_start(
                rw_sb[:].rearrange("p (g c) -> p g c", g=C),
                d_rw.ap().rearrange("g p c -> p g c"))
            perm_sb = wp.tile([128, C * 128], F32)
            nc.sync.dma_start(
                perm_sb[:].rearrange("p (g c) -> p g c", g=C),
                d_perm.ap().rearrange("g p c -> p g c"))
            m1_sb = wp.tile([2, 2 * A], F8)
            nc.sync.dma_start(m1_sb[:], d_m1[:])
            b1_sb = wp.tile([128, AK], F32)
            nc.sync.dma_start(b1_sb[:], d_b1[:])
            b2a_sb = wp.tile([128, HK], F32)
            nc.sync.dma_start(b2a_sb[:], d_b2a[:])
            b2b_sb = wp.tile([128, HK], F32)
            nc.sync.dma_start(b2b_sb[:], d_b2b[:])
            xt8_sb = wp.tile([128, HKP * 2 * TOK], F8, tag="xt8")
            nc.sync.dma_start(
                xt8_sb[:].rearrange("p (hp c) -> p hp c", hp=HKP),
                d_xt8.ap().rearrange("hp p c -> p hp c"))
            fc1w_sb = wp.tile([128, HKP * 2 * A], F8, tag="fc1w")
            nc.sync.dma_start(
                fc1w_sb[:].rearrange("p (hp c) -> p hp c", hp=HKP),
                d_fc1w.ap().rearrange("hp p c -> p hp c"))
            x16_sb = wp.tile([128, HK * TOK], F16, tag="x16")
            nc.sync.dma_start(
                x16_sb[:].rearrange("p (hk c) -> p hk c", hk=HK),
                d_x16.ap().rearrange("hk p c -> p hk c"))
            fc2w_sb = wp.tile([128, AKP * 2 * H], F8, tag="fc2w")
            nc.sync.dma_start(
                fc2w_sb[:].rearrange("p (ap c) -> p ap c", ap=AKP),
                d_fc2w.ap().rearrange("ap p c -> p ap c"))

            # constants
            ones1_16 = wp.tile([1, 128], F16)
            nc.gpsimd.memset(ones1_16[:], 1.0)
            ones128 = wp.tile([128, 1], F32)
            nc.gpsimd.memset(ones128[:], 1.0)
            inv_an2 = wp.tile([1, 128], F32)
            nc.gpsimd.memset(inv_an2[:], 1.0 / (an * an))
            one_row = wp.tile([1, 128], F32)
            nc.gpsimd.memset(one_row[:], 1.0)
            vt2_sb = wp.tile([2, 2 * TOK], F8)
            nc.gpsimd.memset(vt2_sb[:], 0.0)

            # z1 lives across fc1->fc2
            z1_sb = wp.tile([128, AK * TOK], F8, tag="z1")

            # ---------------- phase 1: sem ([s,30] per slot) -------------
            sem_own = rp.tile([128, NSEM * NC30], F32)
            for grp in range(3):
                ps = pp.tile([128, 6 * NC30], F32, tag="sm", bufs=1,
                             name=f"ps_sem_{grp}")
                for jj in range(6):
                    j = grp * 6 + jj
                    dst = ps[:, jj * NC30:(jj + 1) * NC30]
                    base = grp * HK * 6 * S
                    for hk in range(HK):
                        nc.tensor.matmul(
                            dst,
                            xtsem_sb[:, base + hk * 6 * S + jj * S:
                                     base + hk * 6 * S + (jj + 1) * S],
                            sw_sb[:, hk * NC30:(hk + 1) * NC30],
                            start=(hk == 0), stop=False)
                    nc.tensor.matmul(dst, ones1_16[:], semb_sb[:],
                                     start=False, stop=True)
                nc.scalar.copy(
                    sem_own[:, grp * 6 * NC30:(grp + 1) * 6 * NC30], ps[:])

            # ---------------- phase 2: squash + sem_pair -----------------
            sem2 = rp.tile([128, NSEM * NC30], F32)
            nc.vector.tensor_tensor(sem2[:], sem_own[:], sem_own[:], op=ALU.mult)
            sqt = rp.tile([128, NSEM * C], F32)
            nc.vector.tensor_reduce(
                sqt[:].rearrange("p (slot cc) -> p slot cc", cc=C),
                sem2[:].rearrange("p (slot n cc) -> p slot cc n", n=N, cc=C),
                axis=mybir.AxisListType.X, op=ALU.add)
            lnq = rp.tile([128, NSEM * C], F32)
            nc.scalar.activation(lnq[:], sqt[:], AF.Ln)
            sqq = rp.tile([128, NSEM * C], F32)
            nc.scalar.activation(sqq[:], lnq[:], AF.Exp, scale=0.5)
            up = rp.tile([128, NSEM * C], F32)
            nc.vector.tensor_scalar_add(up[:], sqt[:], 1.0)
            ru = rp.tile([128, NSEM * C], F32)
            nc.vector.reciprocal(ru[:], up[:])
            fq = rp.tile([128, NSEM * C], F32)
            nc.vector.tensor_tensor(fq[:], sqq[:], ru[:], op=ALU.mult)
            # expand f to (slot, n, c) so the pair gather stays 3-dim
            fq18 = rp.tile([128, NSEM * an * C], F32)
            fqa = fq[:]
            fq_b = bass.AP(fqa.tensor, fqa.offset,
                           [fqa.ap[0], [C, NSEM], [0, an], [1, C]])
            nc.vector.tensor_copy(
                fq18[:].rearrange("p (slot n c) -> p slot n c", n=an, c=C),
                fq_b)

            # sem_pair[p, (pair, n, c)] = sem_own[s, (slot(pair), n, c)] * fq
            spair = rp.tile([128, NPAIR * an * C], F16)
            so = sem_own[:]
            gather = bass.AP(so.tensor, so.offset,
                             [so.ap[0], [NC30, BL], [NC30, C], [1, an * C]])
            f18 = fq18[:]
            fgather = bass.AP(f18.tensor, f18.offset,
                              [f18.ap[0], [an * C, BL], [an * C, C], [1, an * C]])
            nc.vector.tensor_tensor(
                spair[:].rearrange("p (i u nc) -> p i u nc", i=BL, u=C),
                gather, fgather, op=ALU.mult)

            # masked copies (one per rw group g); one rides on Pool
            spg = rp.tile([128, 3 * NPAIR * an * C], F16)
            ms = masks_sb[:]
            for g in range(C):
                mask_b = bass.AP(ms.tensor, ms.offset + g * NPAIR,
                                 [ms.ap[0], [1, NPAIR], [0, an * C]])
                eng = nc.gpsimd if g == 2 else nc.vector
                eng.tensor_tensor(
                    spg[:, g * NPAIR * an * C:(g + 1) * NPAIR * an * C]
                    .rearrange("p (pair nc) -> p pair nc", nc=an * C),
                    spair[:].rearrange("p (pair nc) -> p pair nc", nc=an * C),
                    mask_b, op=ALU.mult)

            if dbg:
                nc.sync.dma_start(d_dsem[:], sem_own[:])
                nc.sync.dma_start(d_dsp[:], spair[:])

            # ---------------- phase 3: priors [d, (n, pair)] -------------
            ps_pr = pp.tile([128, ANP], F32, tag="pr", name="ps_pr")
            spg_ap = spg[:]
            for n in range(an):
                first = True
                for g in range(C):
                    for cc in range(C):
                        mov = bass.AP(
                            spg_ap.tensor,
                            spg_ap.offset + g * NPAIR * an * C + n * C + cc,
                            [spg_ap.ap[0], [an * C, NPAIR]])
                        nc.tensor.matmul(
                            ps_pr[:, n * NPAIR:(n + 1) * NPAIR],
                            rw_sb[:, g * an * C * S + n * C * S + cc * S:
                                  g * an * C * S + n * C * S + (cc + 1) * S],
                            mov,
                            start=first, stop=(g == C - 1 and cc == C - 1))
                        first = False

            # priors to SBUF once: routing reads never touch two PSUMs
            pr_sb = rp.tile([128, ANP], F32)
            nc.scalar.copy(pr_sb[:], ps_pr[:])
            if dbg:
                nc.sync.dma_start(d_dpr[:], pr_sb[:])

            # ---------------- phase 4: routing ---------------------------
            # priors view [p partitions(d), pair, n] with n innermost
            pr = pr_sb[:]
            pr_pn = bass.AP(pr.tensor, pr.offset,
                            [pr.ap[0], [1, NPAIR], [NPAIR, an]])

            vote = rp.tile([128, NPAIR], F32)
            scr48 = rp.tile([128, NPAIR], F32)
            outv = rp.tile([128, NPAIR], F32)
            scr288 = rp.tile([128, ANP], F32)
            lv = rp.tile([1, NPAIR], F32)
            sv = rp.tile([1, NPAIR], F32)
            uv = rp.tile([1, NPAIR], F32)
            rv = rp.tile([1, NPAIR], F32)
            fv = rp.tile([1, NPAIR], F32)
            mx = rp.tile([1, NPAIR], F32)
            l2 = rp.tile([1, ANP], F32)
            ex = rp.tile([1, ANP], F32)
            es = rp.tile([1, NPAIR], F32)
            ers = rp.tile([1, NPAIR], F32)
            probs16 = rp.tile([1, ANP], F16)
            logits = rp.tile([1, ANP], F32)

            def squash_vote(it):
                # f = sqrt(sq)/(1+sq); it0 vote is an*true -> rescale consts
                nc.vector.tensor_tensor(scr48[:], vote[:], vote[:], op=ALU.mult)
                ps_sq = pp.tile([1, NPAIR], F32, tag="sm", bufs=1,
                                name=f"ps_sq_{it}")
                nc.tensor.matmul(ps_sq[:], ones128[:], scr48[:],
                                 start=True, stop=True)
                nc.scalar.activation(lv[:], ps_sq[:], AF.Ln)
                nc.scalar.activation(sv[:], lv[:], AF.Exp, scale=0.5)
                if it == 0:
                    nc.vector.tensor_scalar(uv[:], ps_sq[:],
                                            1.0 / (an * an), 1.0,
                                            op0=ALU.mult, op1=ALU.add)
                else:
                    nc.vector.tensor_scalar_add(uv[:], ps_sq[:], 1.0)
                nc.vector.reciprocal(rv[:], uv[:])
                nc.vector.tensor_tensor(fv[:], sv[:], rv[:], op=ALU.mult)
                ps_fr = pp.tile([128, NPAIR], F32, tag="sm", bufs=1,
                                name=f"ps_fr_{it}")
                rep = inv_an2 if it == 0 else one_row
                nc.tensor.matmul(ps_fr[:], rep[:], fv[:], start=True, stop=True)
                nc.vector.tensor_tensor(outv[:], vote[:], ps_fr[:], op=ALU.mult)

            def deltas(it):
                ov = outv[:]
                ov_b = bass.AP(ov.tensor, ov.offset,
                               [ov.ap[0], [0, an], [1, NPAIR]])
                nc.vector.tensor_tensor(
                    scr288[:].rearrange("p (n pair) -> p n pair", n=an),
                    pr_sb[:].rearrange("p (n pair) -> p n pair", n=an),
                    ov_b, op=ALU.mult)
                ps_d = pp.tile([1, ANP], F32, tag="sm", bufs=1,
                               name=f"ps_d_{it}")
                nc.tensor.matmul(ps_d[:], ones128[:], scr288[:],
                                 start=True, stop=True)
                if it == 0:
                    nc.scalar.copy(logits[:], ps_d[:])
                else:
                    nc.vector.tensor_tensor(logits[:], logits[:], ps_d[:],
                                            op=ALU.add)

            def softmax_probs(it):
                dl = logits[:]
                dl_pn = bass.AP(dl.tensor, dl.offset,
                                [dl.ap[0], [1, NPAIR], [NPAIR, an]])
                nc.vector.tensor_reduce(mx[:], dl_pn,
                                        axis=mybir.AxisListType.X, op=ALU.max)
                mxa = mx[:]
                mx_b = bass.AP(mxa.tensor, mxa.offset,
                               [mxa.ap[0], [0, an], [1, NPAIR]])
                nc.vector.tensor_tensor(
                    l2[:].rearrange("p (n pair) -> p n pair", n=an),
                    dl.rearrange("p (n pair) -> p n pair", n=an),
                    mx_b, op=ALU.subtract)
                nc.scalar.activation(ex[:], l2[:], AF.Exp)
                exa = ex[:]
                ex_pn = bass.AP(exa.tensor, exa.offset,
                                [exa.ap[0], [1, NPAIR], [NPAIR, an]])
                nc.vector.tensor_reduce(es[:], ex_pn,
                                        axis=mybir.AxisListType.X, op=ALU.add)
                nc.vector.reciprocal(ers[:], es[:])
                ersa = ers[:]
                ers_b = bass.AP(ersa.tensor, ersa.offset,
                                [ersa.ap[0], [0, an], [1, NPAIR]])
                nc.vector.tensor_tensor(
                    probs16[:].rearrange("p (n pair) -> p n pair", n=an),
                    ex[:].rearrange("p (n pair) -> p n pair", n=an),
                    ers_b, op=ALU.mult)

            def vote_from_probs(it):
                ps_pb = pp.tile([128, ANP], F32, tag="sm", bufs=1,
                                name=f"ps_pb_{it}")
                nc.tensor.matmul(ps_pb[:], ones1_16[:], probs16[:],
                                 start=True, stop=True)
                nc.vector.tensor_tensor(scr288[:], pr_sb[:], ps_pb[:],
                                        op=ALU.mult)
                sc = scr288[:]
                sc_pn = bass.AP(sc.tensor, sc.offset,
                                [sc.ap[0], [1, NPAIR], [NPAIR, an]])
                nc.vector.tensor_reduce(vote[:], sc_pn,
                                        axis=mybir.AxisListType.X, op=ALU.add)

            # iter 0: probs uniform = 1/an (vote holds an*true vote)
            nc.vector.tensor_reduce(vote[:], pr_pn,
                                    axis=mybir.AxisListType.X, op=ALU.add)
            squash_vote(0)
            deltas(0)
            # iter 1
            softmax_probs(1)
            vote_from_probs(1)
            squash_vote(1)
            deltas(1)
            # iter 2 (final vote, unsquashed)
            softmax_probs(2)
            vote_from_probs(2)

            if dbg:
                nc.sync.dma_start(d_dvote[:], vote[:])

            # ---------------- phase 5: vote -> vt2 (fp8, DR layout) ------
            # Reference h_out[b,s,c] = vote_flat[384b + 3s + c]: pair row
            # 3i+u with u=(3s+c)//128, token s'=(3s+c)%128.  Per (c,u):
            # transpose vote columns {3i+u} through the permutation
            # s'=(3s+c)%128, take the u-segment of s, quantize into
            # vtI[i, c*128+s]; then 2 tiny partition-merging DMAs -> vt2.
            vtI = rp.tile([BL, C * S], F8)
            va = vote[:]
            for c in range(C):
                for u in range(C):
                    s0 = (128 * u - c + 2) // 3 if u > 0 else 0
                    s1 = (128 * (u + 1) - c + 2) // 3
                    s1 = min(s1, S)
                    if s1 <= s0:
                        continue
                    lhsT = bass.AP(va.tensor, va.offset + u,
                                   [va.ap[0], [3, BL]])
                    ps_t = pp.tile([BL, S], F32, tag="sm", bufs=1,
                                   name=f"ps_t_{c}_{u}")
                    nc.tensor.matmul(ps_t[:], lhsT,
                                     perm_sb[:, c * 128:(c + 1) * 128],
                                     is_transpose=True)
                    eng = nc.scalar if (c * C + u) % 2 == 0 else None
                    if eng is not None:
                        nc.scalar.activation(
                            vtI[:, c * S + s0:c * S + s1],
                            ps_t[:, s0:s1], AF.Copy, scale=4.0)
                    else:
                        nc.vector.tensor_scalar_mul(
                            vtI[:, c * S + s0:c * S + s1],
                            ps_t[:, s0:s1], 4.0)
            # vt2[p_row][ksub*TOK + i*128 + s] = vtI[i, (2*ksub+p_row)*128+s]
            vI = vtI[:]
            for p_row in range(2):
                for ksub in range(2):
                    cc = 2 * ksub + p_row
                    if cc >= C:
                        continue  # row (1,1) stays zero
                    src = bass.AP(vI.tensor, vI.offset + cc * S,
                                  [vI.ap[0], [1, S]])
                    nc.gpsimd.dma_start(
                        vt2_sb[p_row:p_row + 1,
                               ksub * TOK:(ksub + 1) * TOK]
                        .rearrange("p (i s) -> p i s", i=BL),
                        src)

            # ---------------- phase 6: fc1 (+m1 term) --------------------
            f1 = fc1w_sb[:]
            x8 = xt8_sb[:]
            m1a = m1_sb[:]
            v2a = vt2_sb[:]
            for ak in range(AK):
                for half in range(2):
                    bi = ak * 2 + half
                    ps_z = pp.tile([128, 1024], F32, tag="z1", bufs=3,
                                   name=f"ps_z_{bi}")
                    for tb4 in range(4):
                        tb = half * 4 + tb4
                        dst = ps_z[:, tb4 * 256:(tb4 + 1) * 256]
                        for hp in range(HKP):
                            lhsT = bass.AP(f1.tensor,
                                           f1.offset + hp * 2 * A + ak * 128,
                                           [f1.ap[0], [A, 2], [1, 128]])
                            rhs = bass.AP(x8.tensor,
                                          x8.offset + hp * 2 * TOK + tb * 256,
                                          [x8.ap[0], [TOK, 2], [1, 256]])
                            nc.tensor.matmul(dst, lhsT, rhs, perf_mode=DR,
                                             start=(hp == 0), stop=False)
                        lhsT = bass.AP(m1a.tensor, m1a.offset + ak * 128,
                                       [m1a.ap[0], [A, 2], [1, 128]])
                        rhs = bass.AP(v2a.tensor, v2a.offset + tb * 256,
                                      [v2a.ap[0], [TOK, 2], [1, 256]])
                        nc.tensor.matmul(dst, lhsT, rhs, perf_mode=DR,
                                         start=False, stop=True)
                    zoff = ak * TOK + half * 1024
                    zdst = z1_sb[:, zoff:zoff + 1024]
                    if bi % 2 == 0:
                        nc.scalar.activation(zdst, ps_z[:], AF.Relu,
                                             bias=b1_sb[:, ak:ak + 1])
                    else:
                        nc.vector.tensor_scalar(zdst, ps_z[:],
                                                b1_sb[:, ak:ak + 1], 0.0,
                                                op0=ALU.add, op1=ALU.max)

            if dbg:
                nc.sync.dma_start(d_dz1[:], z1_sb[:])

            # ---------------- phase 7: fc2 + residual + out --------------
            f2 = fc2w_sb[:]
            z1a = z1_sb[:]
            for hk in range(HK):
                for half in range(2):
                    bi = hk * 2 + half
                    ps_f = pp.tile([128, 1024], F32, tag="z1", bufs=3,
                                   name=f"ps_f_{bi}")
                    for tb4 in range(4):
                        tb = half * 4 + tb4
                        dst = ps_f[:, tb4 * 256:(tb4 + 1) * 256]
                        for ap_ in range(AKP):
                            lhsT = bass.AP(f2.tensor,
                                           f2.offset + ap_ * 2 * H + hk * 128,
                                           [f2.ap[0], [H, 2], [1, 128]])
                            rhs = bass.AP(z1a.tensor,
                                          z1a.offset + 2 * ap_ * TOK + tb * 256,
                                          [z1a.ap[0], [TOK, 2], [1, 256]])
                            nc.tensor.matmul(dst, lhsT, rhs, perf_mode=DR,
                                             start=(ap_ == 0),
                                             stop=(ap_ == AKP - 1))
                    xs = x16_sb[:, hk * TOK + half * 1024:
                                hk * TOK + (half + 1) * 1024]
                    ot = sp.tile([128, 1024], F16, tag="ot", bufs=2,
                                 name=f"ot_{bi}")
                    h16 = sp.tile([128, 1024], F16, tag="h16", bufs=2,
                                  name=f"h16_{bi}")
                    nc.scalar.activation(h16[:], ps_f[:], AF.Relu,
                                         bias=b2a_sb[:, hk:hk + 1],
                                         scale=1.0 / 256.0)
                    eng = nc.gpsimd if bi in (1, 3, 5, 7) else nc.vector
                    eng.tensor_tensor(ot[:], h16[:], xs, op=ALU.add)
                    nc.sync.dma_start(
                        d_out.ap()[hk][:, half * 1024:(half + 1) * 1024],
                        ot[:])

    nc.compile()
    return nc


# ---------------------------------------------------------------------------
# host marshaling
# ---------------------------------------------------------------------------

def _prep_core_inputs(k, x, shared, act_n):
    an = act_n
    own = np.array([(48 * k + 43 * i) % B for i in range(BL)])
    sem_ex = np.array([(16 * k + j) % B for j in range(NSEM)])

    # xtsem[grp, hk, p, jj*S+s] = x[sem_ex[6*grp+jj], s, hk*128+p]  (fp16)
    xs = _f16(np.transpose(x[sem_ex], (2, 0, 1)).reshape(H, NSEM_S))
    xtsem = np.ascontiguousarray(
        xs.reshape(HK, 128, 3, 6 * S).transpose(2, 0, 1, 3))

    # masks[p_any, g*NPAIR + pair] = 1 if rw group of pair == g
    masks = np.zeros((3, NPAIR), np.float32)
    for i in range(BL):
        for u in range(C):
            t = 3 * int(own[i]) + u
            masks[t // B, 3 * i + u] = 1.0
    masks_rep = _f16(np.broadcast_to(masks.reshape(1, 3 * NPAIR), (128, 3 * NPAIR)))

    # xt8[hp, p, ksub*TOK + e*S+s] = x[own[e], s, (2hp+ksub)*128+p]  (fp8)
    xo = np.transpose(x[own], (2, 0, 1)).reshape(H, TOK)  # [h, tok]
    xt8 = _f8(xo.reshape(HKP, 2, 128, TOK).transpose(0, 2, 1, 3)
              .reshape(HKP, 128, 2 * TOK))

    # x16[hk, p, tok]
    x16 = _f16(xo).reshape(HK, 128, TOK)

    return {
        "xtsem": xtsem,
        "masks": masks_rep,
        "xt8": xt8,
        "x16": x16,
        **{n: shared[n] for n in ("sw", "semb", "rw", "perm", "m1", "b1",
                                  "b2a", "b2b", "fc1w", "fc2w")},
    }


_CACHE = {}


def _make_shared(inputs):
    fc1_w = np.asarray(inputs["fc1_w"], np.float32)
    fc1_b = np.asarray(inputs["fc1_b"], np.float32)
    fc2_w = np.asarray(inputs["fc2_w"], np.float32)
    fc2_b = np.asarray(inputs["fc2_b"], np.float32)
    efc1 = np.asarray(inputs["efc1"], np.float32)
    efc2 = np.asarray(inputs["efc2"], np.float32)
    sem_w = np.asarray(inputs["sem_w"], np.float32)
    sem_b = np.asarray(inputs["sem_b"], np.float32)
    route_weights = np.asarray(inputs["route_weights"], np.float32)
    larger_w = np.asarray(inputs["larger_w"], np.float32)
    larger_b = np.asarray(inputs["larger_b"], np.float32)
    elarger = np.asarray(inputs["elarger"], np.float32)
    t = int(np.asarray(inputs["t"]))
    sf = np.float32(int(np.asarray(inputs["s"])))
    an = t + 1

    gfc1 = _sigmoid_f32(sf * efc1[t])
    gfc2 = _sigmoid_f32(sf * efc2[t])
    glarger = _sigmoid_f32(sf * elarger[t])

    lwg = larger_w * glarger[None, :]
    lb_eff = larger_b * glarger
    m1 = lwg @ fc1_w                                  # [C, A]
    b1 = (fc1_b + lb_eff @ fc1_w).astype(np.float32)  # [A]
    w2 = fc2_w * gfc1[:, None] * gfc2[None, :]
    b2 = (fc2_b * gfc2).astype(np.float32)            # [H]

    # m1 packed for DoubleRow with K=4 (rows: c=2*ksub+p, row3 zero), x4 scale
    m14 = np.zeros((2, 2, A), np.float32)
    for c in range(C):
        m14[c % 2, c // 2] = 4.0 * m1[c]

    # fc1w[hp, p, ksub*A + a] = 16*fc1_w[(2hp+ksub)*128+p, a]
    fc1w8 = _f8((16.0 * fc1_w).reshape(HKP, 2, 128, A).transpose(0, 2, 1, 3)
                .reshape(HKP, 128, 2 * A))
    # fc2w[ap, p, ksub*H + h] = 16*w2[(2ap+ksub)*128+p, h]
    fc2w8 = _f8((16.0 * w2).reshape(AKP, 2, 128, H).transpose(0, 2, 1, 3)
                .reshape(AKP, 128, 2 * H))

    # rw[g, s, (n, c, d)] = route_weights[g, n, s*C+c -> (s, c), d]  fp16
    rw4 = route_weights.reshape(C, N, S, C, S)
    rw16 = np.stack([
        _f16(np.transpose(rw4[g, :an], (1, 0, 2, 3)).reshape(S, an * C * S))
        for g in range(C)])

    # perm[c][s', s] = 1 iff s' == (3s+c) % 128
    perm = np.zeros((C, S, S), np.float32)
    for c in range(C):
        s = np.arange(S)
        perm[c, (3 * s + c) % S, s] = 1.0

    return {
        "sw": _f16(np.transpose(sem_w, (1, 0, 2)).reshape(H, NC30)
                   ).reshape(HK, 128, NC30),
        "semb": _f16(sem_b.reshape(1, NC30)),
        "rw": rw16,
        "perm": perm,
        "m1": _f8(m14.reshape(2, 2 * A)),
        "b1": np.ascontiguousarray((16.0 * b1).reshape(AK, 128).T),
        "b2a": np.ascontiguousarray(b2.reshape(HK, 128).T),
        "b2b": np.ascontiguousarray((256.0 * b2).reshape(HK, 128).T),
        "fc1w": fc1w8,
        "fc2w": fc2w8,
    }


def kernel(**inputs):
    x = np.asarray(inputs["x"], np.float32)
    t = int(np.asarray(inputs["t"]))
    an = t + 1
    shared = _make_shared(inputs)

    if an not in _CACHE:
        _CACHE[an] = _build_program(an)
    nc = _CACHE[an]

    in_maps = [_prep_core_inputs(k, x, shared, an) for k in range(NCORES)]
    res = bass_utils.run_bass_kernel_spmd(nc, in_maps, core_ids=list(range(NCORES)))
    out = np.empty((B, S, H), np.float32)
    for k in range(NCORES):
        own = [(48 * k + 43 * i) % B for i in range(BL)]
        # out dram [HK, 128, TOK] f16: [hk, p, e*S+s] -> x[own[e], s, hk*128+p]
        oc = np.asarray(res.results[k]["out"], np.float32)
        oc = oc.reshape(HK, 128, BL, S).transpose(2, 3, 0, 1).reshape(BL, S, H)
        out[own] = oc
    return out
